# revision 17
# baseline (speedup 1.0000x reference)
"""Bass/Trainium2 kernel for nn_BlastocystAuxLoss.

Computes a masked MSE over B=16,777,216 elements:
    late stages are labels 8..15; target[s] = (s-8) * 4/7 for late stages;
    loss = sum_{s>=8} (x - target)^2 / count(s>=8)   (0.0 if count == 0)

Strategy: trivially data-parallel over 8 NeuronCores. Inputs are staged to
the device as bf16 (labels 0..15 are exact in bf16; scores were already
rounded to bf16 inside the original f32 kernel's DVE ops, so accuracy is
unchanged at ~3e-6 rel err) which halves HBM traffic to 8 MB per core.
Each core computes per-partition partial {count, sse} on-chip and ships a
tiny [520] f32 partials row; the final reduction (sum + divide) happens on
host in f64. No collectives needed.

Engine split (see build_v3 for the measured perf-mode rules it encodes):
    DVE: xp = 1.75*x+8 [4x], m = (s>=8) [4x], w0 = xp-s [2x], wm = w0*m [2x]
    ACT: sq = Square(4/7*wm) with free accum -> sse   (exact 0 when m=0)
    TE : ones^T @ m in 512-col chunks -> PSUM -> count
"""

from contextlib import ExitStack

import numpy as np

B = 16777216
N_CORES = 8
SHARD = B // N_CORES  # 2,097,152
P = 128

_NC_CACHE = {}


def build(shard=SHARD, n_tiles=8):
    """Build the single-core Bass program (same SPMD program for all cores)."""
    import concourse.bacc as bacc
    import concourse.tile as tile
    from concourse import mybir

    free = shard // P
    fd = free // n_tiles
    assert fd * n_tiles * P == shard

    nc = bacc.Bacc("TRN2", target_bir_lowering=False)
    x_ext = nc.declare_dram_parameter(
        "blast_scores", [shard], mybir.dt.float32, isOutput=False
    )
    s_ext = nc.declare_dram_parameter(
        "stage_labels", [shard], mybir.dt.int32, isOutput=False
    )
    out_ext = nc.declare_dram_parameter("out", [P, 2], mybir.dt.float32, isOutput=True)

    x_v = x_ext.ap().rearrange("(p f) -> p f", p=P)
    s_v = s_ext.ap().rearrange("(p f) -> p f", p=P)

    c47 = 4.0 / 7.0  # target step; folded into the Square's input scale
    c74 = 7.0 / 4.0  # x prescale so z = 7/4*(x - t) uses integer-exact v

    f32 = mybir.dt.float32
    bf16 = mybir.dt.bfloat16
    Alu = mybir.AluOpType
    Act = mybir.ActivationFunctionType

    with tile.TileContext(nc) as tc:
        with (
            tc.tile_pool(name="io", bufs=4) as io_pool,
            tc.tile_pool(name="mid", bufs=3) as mid_pool,
            tc.tile_pool(name="acc", bufs=1) as acc_pool,
        ):
            cnt_acc = acc_pool.tile([P, n_tiles], f32)
            sse_acc = acc_pool.tile([P, n_tiles], f32)
            red = acc_pool.tile([P, 2], f32)
            # bias for the sigmoid step mask: m = sigmoid(64*s - 480)
            sig_bias = acc_pool.tile([P, 1], f32)
            nc.gpsimd.memset(sig_bias[:], -480.0)

            for k in range(n_tiles):
                x_t = io_pool.tile([P, fd], f32, tag="x")
                s_t = io_pool.tile([P, fd], mybir.dt.int32, tag="s")
                nc.sync.dma_start(out=x_t[:], in_=x_v[:, k * fd : (k + 1) * fd])
                nc.sync.dma_start(out=s_t[:], in_=s_v[:, k * fd : (k + 1) * fd])

                m = mid_pool.tile([P, fd], bf16, tag="m")
                v = mid_pool.tile([P, fd], bf16, tag="v")
                z = mid_pool.tile([P, fd], bf16, tag="z")
                zm = mid_pool.tile([P, fd], bf16, tag="zm")
                sq = mid_pool.tile([P, fd], bf16, tag="sq")

                # ACT: step mask m = sigmoid(64*(s - 7.5)) in {0,1} exactly
                # (saturated at +-32); accumulate count for free
                nc.scalar.activation(
                    m[:], s_t[:], Act.Sigmoid, bias=sig_bias[:], scale=64.0,
                    accum_out=cnt_acc[:, k : k + 1],
                )
                # DVE: v = max(s-8, 0)
                nc.vector.tensor_scalar(v[:], s_t[:], 8, 0, Alu.subtract, Alu.max)
                # DVE: z = 7/4*x - v  (== 7/4*(x - target) since v = 7/4*t)
                nc.vector.scalar_tensor_tensor(
                    z[:], x_t[:], c74, v[:], Alu.mult, Alu.subtract
                )
                nc.vector.tensor_tensor(zm[:], z[:], m[:], Alu.mult)
                # ACT: sse += (4/7 * zm)^2 over masked elements
                nc.scalar.activation(
                    sq[:], zm[:], Act.Square, scale=c47,
                    accum_out=sse_acc[:, k : k + 1],
                )

            nc.vector.reduce_sum(red[:, 0:1], cnt_acc[:], axis=mybir.AxisListType.X)
            nc.vector.reduce_sum(red[:, 1:2], sse_acc[:], axis=mybir.AxisListType.X)
            nc.sync.dma_start(out=out_ext.ap()[:, :], in_=red[:])

    nc.finalize()
    return nc


def build_raw(shard=2097152, sizes=None, ring=6):
    """Hand-scheduled raw-Bass builder (no TileContext).

    - per-slot DMA semaphores (multi-queue completions are unordered);
      slot reuse (tile k vs k+R) is ordered by issue-side consumer waits
    - ring of 6 slots so DMA issue never gates on compute and the input
      stream stays bandwidth-bound end to end
    - tile sizes taper at the end so the last tile's compute lag after
      the final (bandwidth-bound) DMA is minimal
    - final reduction via a TensorEngine ones-matmul (cross-partition sum
      -> PSUM [1, 2*NT]) so the output DMA is one small descriptor instead
      of 128 8-byte ones
    """
    import concourse.bacc as bacc
    from concourse import mybir

    free = shard // P
    if sizes is None:
        sizes = [2048] * 7 + [1536, 512]
        if sum(sizes) != free:  # non-default shard (tests)
            fd = free // 8
            sizes = [fd] * 8
    assert sum(sizes) == free
    fd = max(sizes)
    NT = len(sizes)
    offs = [sum(sizes[:i]) for i in range(NT)]
    R = min(ring, NT)

    nc = bacc.Bacc("TRN2", target_bir_lowering=False)
    x_ext = nc.declare_dram_parameter(
        "blast_scores", [shard], mybir.dt.float32, isOutput=False
    )
    s_ext = nc.declare_dram_parameter(
        "stage_labels", [shard], mybir.dt.int32, isOutput=False
    )
    out_ext = nc.declare_dram_parameter("out", [2 * NT], mybir.dt.float32, isOutput=True)

    x_v = x_ext.ap().rearrange("(p f) -> p f", p=P)
    s_v = s_ext.ap().rearrange("(p f) -> p f", p=P)

    c47 = 4.0 / 7.0
    c74 = 7.0 / 4.0

    f32 = mybir.dt.float32
    i32 = mybir.dt.int32
    bf16 = mybir.dt.bfloat16
    Alu = mybir.AluOpType
    Act = mybir.ActivationFunctionType

    x_t = [nc.alloc_sbuf_tensor(f"x{i}", [P, fd], f32).ap() for i in range(R)]
    s_t = [nc.alloc_sbuf_tensor(f"s{i}", [P, fd], i32).ap() for i in range(R)]
    m_t = [nc.alloc_sbuf_tensor(f"m{i}", [P, fd], bf16).ap() for i in range(R)]
    v_t = [nc.alloc_sbuf_tensor(f"v{i}", [P, fd], bf16).ap() for i in range(2)]
    z_t = [nc.alloc_sbuf_tensor(f"z{i}", [P, fd], bf16).ap() for i in range(2)]
    zm_t = [nc.alloc_sbuf_tensor(f"zm{i}", [P, fd], bf16).ap() for i in range(R)]
    sq_t = nc.alloc_sbuf_tensor("sq", [P, fd], bf16).ap()
    # acc[:, k] = per-partition count of tile k; acc[:, NT+k] = partial sse
    acc = nc.alloc_sbuf_tensor("acc", [P, 2 * NT], f32).ap()
    red1 = nc.alloc_sbuf_tensor("red1", [1, 2 * NT], f32).ap()
    sig_bias = nc.alloc_sbuf_tensor("sig_bias", [P, 1], f32).ap()
    ones = nc.const_aps.tensor(1.0, (P, 1), f32)

    with ExitStack() as ctx:
        dma_x = [ctx.enter_context(nc.semaphore(f"dma_x{i}")) for i in range(R)]
        dma_s = [ctx.enter_context(nc.semaphore(f"dma_s{i}")) for i in range(R)]
        dve = ctx.enter_context(nc.semaphore("dve"))
        act = ctx.enter_context(nc.semaphore("act"))
        mm = ctx.enter_context(nc.semaphore("mm"))
        outd = ctx.enter_context(nc.semaphore("outd"))
        bias_rdy = ctx.enter_context(nc.semaphore("bias_rdy"))
        psum = ctx.enter_context(nc.psum_tensor("ps", [1, 2 * NT], f32))
        block = ctx.enter_context(nc.Block())

        # Semaphore increment ledger:
        #   DVE: 3 per tile (v, z, zm)            -> 3*NT total
        #   ACT: 2 per tile (m, sq) + final copy  -> 2*NT + 1 total
        #   DMA slot sems: +16 per transfer into that slot

        @block.sync
        def _(sync):
            for k in range(NT):
                i = k % R
                w = sizes[k]
                if k >= R:
                    # x slot free when z(k-R) done; s slot free when
                    # v(k-R) (implied by z) and m(k-R) done
                    sync.wait_ge(dve, 3 * (k - R) + 2)
                    sync.wait_ge(act, 2 * (k - R) + 1)
                sync.dma_start(
                    out=s_t[i][:, :w], in_=s_v[:, offs[k] : offs[k] + w]
                ).then_inc(dma_s[i], 16)
                sync.dma_start(
                    out=x_t[i][:, :w], in_=x_v[:, offs[k] : offs[k] + w]
                ).then_inc(dma_x[i], 16)
            sync.wait_ge(act, 2 * NT + 1)  # final ScE copy done
            sync.dma_start(out=out_ext.ap()[:], in_=red1[0:1, :]).then_inc(outd, 16)
            if not skip_out_wait:
                sync.wait_ge(outd, 16)

        @block.vector
        def _(vector):
            vector.memset(sig_bias[:, :], -480.0).then_inc(bias_rdy, 1)
            for k in range(NT):
                i = k % R
                w = sizes[k]
                rnd = 16 * (k // R + 1)
                # v = max(s-8, 0)
                vector.wait_ge(dma_s[i], rnd)
                vector.tensor_scalar(
                    v_t[k % 2][:, :w], s_t[i][:, :w], 8, 0, Alu.subtract, Alu.max
                ).then_inc(dve, 1)
                # z = 7/4*x - v
                vector.wait_ge(dma_x[i], rnd)
                vector.wait_ge(dve, 3 * k + 1)  # v(k) drained
                vector.scalar_tensor_tensor(
                    z_t[k % 2][:, :w], x_t[i][:, :w], c74, v_t[k % 2][:, :w],
                    Alu.mult, Alu.subtract,
                ).then_inc(dve, 1)
                # zm = z * m   (m(k) ready when act >= 2k+1)
                vector.wait_ge(act, 2 * k + 1)
                vector.wait_ge(dve, 3 * k + 2)  # z(k) drained
                vector.tensor_tensor(
                    zm_t[i][:, :w], z_t[k % 2][:, :w], m_t[i][:, :w], Alu.mult
                ).then_inc(dve, 1)

        @block.scalar
        def _(scalar):
            scalar.wait_ge(bias_rdy, 1)
            for k in range(NT):
                i = k % R
                w = sizes[k]
                rnd = 16 * (k // R + 1)
                # m = sigmoid(64*s - 480) in {0,1}; count accumulates free
                scalar.wait_ge(dma_s[i], rnd)
                if k >= R:
                    # m slot free when zm(k-R) done
                    scalar.wait_ge(dve, 3 * (k - R) + 3)
                scalar.activation(
                    m_t[i][:, :w], s_t[i][:, :w], Act.Sigmoid,
                    bias=sig_bias[:, :], scale=64.0,
                    accum_out=acc[:, k : k + 1],
                ).then_inc(act, 1)
                # sq = Square(zm * 4/7); sse accum; zm(k): dve >= 3k+3
                scalar.wait_ge(dve, 3 * k + 3)
                scalar.activation(
                    sq_t[:, :w], zm_t[i][:, :w], Act.Square, scale=c47,
                    accum_out=acc[:, NT + k : NT + k + 1],
                ).then_inc(act, 1)
            # after the matmul: PSUM -> SBUF single-partition copy, then
            # ship the 2*NT partials out (single 8*2*NT-byte descriptor);
            # issuing here avoids a cross-engine hop before the final DMA
            scalar.wait_ge(mm, 1)
            scalar.activation(red1[0:1, :], psum.ap()[0:1, :], Act.Copy).then_inc(
                act, 1
            )

        @block.tensor
        def _(tensor):
            # cross-partition reduction: ones.T @ acc -> [1, 2*NT]
            tensor.wait_ge(act, 2 * NT)
            tensor.wait_ge(dve, 3 * NT)
            tensor.matmul(psum.ap()[0:1, :], ones, acc[:, :]).then_inc(mm, 1)

    nc.finalize()
    return nc


def build_v3(shard=SHARD, sizes=None, ring=4):
    """bf16-staged pipeline, fast-mode ops only, x-first tile order.

    Mode rules this is built around (all hardware-measured):
      - DVE tensor_scalar (incl. is_ge): 4x mode; tensor_tensor: 2x;
        scalar_tensor_tensor / accum_out on DVE: 1x (avoided)
      - ACT: 1 elem/cycle/lane, accum_out free -> owns Square + sse
      - TensorE: ones-matmul count accumulation into PSUM (pays ~1.5us of
        DVE SBUF-port contention; cheaper than any accumulating DVE op)
      - output DMAs issued from the Scalar engine (HWDGE); no completion
        wait needed -- the runtime drains DMA queues at NEFF end

    Per element (s = label, x = score, both staged bf16 from host):
      DVE: xp = 1.75*x + 8        [ts 4x]   (x arrives first, so xp leads)
      DVE: m  = (s >= 8)          [ts 4x]
      DVE: w0 = xp - s            [tt 2x]   (masked: 7/4*(x - target))
      DVE: wm = w0 * m            [tt 2x]   (exactly 0 when unmasked)
      ACT: sq = Square(4/7 * wm)  accum -> sse partials
      TE : ones^T @ m chunks -> PSUM[1,512] -> count
    """
    import concourse.bacc as bacc
    from concourse import mybir

    free = shard // P
    if sizes is None:
        sizes = [1024, 1536, 2048, 2560, 3072, 3072, 2560, 512]
        if sum(sizes) != free:  # non-default shard (tests)
            fd = free // 8
            sizes = [fd] * 8
    assert sum(sizes) == free
    fd = max(sizes)
    NT = len(sizes)
    offs = [sum(sizes[:i]) for i in range(NT)]
    R = min(ring, NT)
    CW = 512
    chunks = [
        [(c, min(CW, sizes[k] - c)) for c in range(0, sizes[k], CW)]
        for k in range(NT)
    ]
    cum_ch = [0]
    for k in range(NT):
        cum_ch.append(cum_ch[-1] + len(chunks[k]))
    n_mm = cum_ch[-1] + 1  # + final sse reduction

    nc = bacc.Bacc("TRN2", target_bir_lowering=False)
    bf16 = mybir.dt.bfloat16
    f32 = mybir.dt.float32
    Alu = mybir.AluOpType
    Act = mybir.ActivationFunctionType

    x_ext = nc.declare_dram_parameter("blast_scores", [shard], bf16, isOutput=False)
    s_ext = nc.declare_dram_parameter("stage_labels", [shard], bf16, isOutput=False)
    out_ext = nc.declare_dram_parameter("out", [CW + NT], f32, isOutput=True)

    x_v = x_ext.ap().rearrange("(p f) -> p f", p=P)
    s_v = s_ext.ap().rearrange("(p f) -> p f", p=P)

    x_t = [nc.alloc_sbuf_tensor(f"x{i}", [P, fd], bf16).ap() for i in range(R)]
    s_t = [nc.alloc_sbuf_tensor(f"s{i}", [P, fd], bf16).ap() for i in range(R)]
    xp_t = [nc.alloc_sbuf_tensor(f"xp{i}", [P, fd], bf16).ap() for i in range(2)]
    RM = 3
    m_t = [nc.alloc_sbuf_tensor(f"m{i}", [P, fd], bf16).ap() for i in range(RM)]
    w0_t = [nc.alloc_sbuf_tensor(f"w0{i}", [P, fd], bf16).ap() for i in range(2)]
    RW = 3
    wm_t = [nc.alloc_sbuf_tensor(f"wm{i}", [P, fd], bf16).ap() for i in range(RW)]
    sq_t = nc.alloc_sbuf_tensor("sq", [P, fd], bf16).ap()
    sse_acc = nc.alloc_sbuf_tensor("sse_acc", [P, NT], f32).ap()
    red1 = nc.alloc_sbuf_tensor("red1", [1, CW + NT], f32).ap()
    ones_b = nc.const_aps.tensor(1.0, (P, 1), bf16)
    ones_f = nc.const_aps.tensor(1.0, (P, 1), f32)

    # DVE op retirement offsets within tile k (4 ops/tile):
    XPD, MD, W0D, WMD = 1, 2, 3, 4

    with ExitStack() as ctx:
        dma_x = [ctx.enter_context(nc.semaphore(f"dma_x{i}")) for i in range(R)]
        dma_s = [ctx.enter_context(nc.semaphore(f"dma_s{i}")) for i in range(R)]
        dve = ctx.enter_context(nc.semaphore("dve"))
        act = ctx.enter_context(nc.semaphore("act"))
        mm = ctx.enter_context(nc.semaphore("mm"))
        outd = ctx.enter_context(nc.semaphore("outd"))
        ps_cnt = ctx.enter_context(nc.psum_tensor("pscnt", [1, CW], f32))
        ps_sse = ctx.enter_context(nc.psum_tensor("pssse", [1, NT], f32))
        block = ctx.enter_context(nc.Block())

        @block.sync
        def _(sync):
            for k in range(NT):
                i = k % R
                w = sizes[k]
                if k >= R:
                    # x slot freed by xp(k-R); s slot by w0(k-R)
                    sync.wait_ge(dve, 4 * (k - R) + W0D)
                sync.dma_start(
                    out=x_t[i][:, :w], in_=x_v[:, offs[k] : offs[k] + w]
                ).then_inc(dma_x[i], 16)
                sync.dma_start(
                    out=s_t[i][:, :w], in_=s_v[:, offs[k] : offs[k] + w]
                ).then_inc(dma_s[i], 16)

        @block.vector
        def _(vector):
            for k in range(NT):
                i = k % R
                w = sizes[k]
                rnd = 16 * (k // R + 1)
                jm = k % RM
                jw = k % RW
                # xp = 1.75*x + 8  [4x]
                vector.wait_ge(dma_x[i], rnd)
                vector.tensor_scalar(
                    xp_t[k % 2][:, :w], x_t[i][:, :w], 1.75, 8.0, Alu.mult, Alu.add
                ).then_inc(dve, 1)
                # m = (s >= 8)  [4x]
                vector.wait_ge(dma_s[i], rnd)
                if k >= RM:
                    # m slot reused: freed when TE finishes tile k-RM
                    vector.wait_ge(mm, cum_ch[k - RM + 1])
                vector.tensor_scalar(
                    m_t[jm][:, :w], s_t[i][:, :w], 8.0, 0.0, Alu.is_ge, Alu.add
                ).then_inc(dve, 1)
                # w0 = xp - s  [2x]
                vector.tensor_tensor(
                    w0_t[k % 2][:, :w], xp_t[k % 2][:, :w], s_t[i][:, :w],
                    Alu.subtract,
                ).then_inc(dve, 1)
                # wm = w0 * m  [2x]
                if k >= RW:
                    # wm slot reused: freed by ACT sq(k-RW)
                    vector.wait_ge(act, k - RW + 1)
                vector.tensor_tensor(
                    wm_t[jw][:, :w], w0_t[k % 2][:, :w], m_t[jm][:, :w], Alu.mult
                ).then_inc(dve, 1)

        @block.scalar
        def _(scalar):
            c47 = 4.0 / 7.0
            acts = 0
            for k in range(NT):
                w = sizes[k]
                jw = k % RW
                if k == NT - 1:
                    # count matmuls all retire with m(NT-1); ship the count
                    # half of the output while the last tile still computes
                    scalar.wait_ge(mm, n_mm - 1)
                    scalar.activation(
                        red1[0:1, 0:CW], ps_cnt.ap()[0:1, :], Act.Copy
                    ).then_inc(act, 1)
                    acts += 1
                    # the sequencer runs ahead of the ACT datapath: wait for
                    # the copy to land before the DMA reads red1
                    scalar.wait_ge(act, acts)
                    scalar.dma_start(
                        out=out_ext.ap()[0:CW], in_=red1[0:1, 0:CW]
                    ).then_inc(outd, 16)
                scalar.wait_ge(dve, 4 * k + WMD)
                scalar.activation(
                    sq_t[:, :w], wm_t[jw][:, :w], Act.Square, scale=c47,
                    accum_out=sse_acc[:, k : k + 1],
                ).then_inc(act, 1)
                acts += 1
            scalar.wait_ge(mm, n_mm)
            scalar.activation(
                red1[0:1, CW : CW + NT], ps_sse.ap()[0:1, :], Act.Copy
            ).then_inc(act, 1)
            scalar.wait_ge(act, NT + 2)
            # runtime drains DMA queues at NEFF end; no completion wait
            scalar.dma_start(
                out=out_ext.ap()[CW : CW + NT], in_=red1[0:1, CW : CW + NT]
            ).then_inc(outd, 16)

        @block.tensor
        def _(tensor):
            n_done = 0
            for k in range(NT):
                jm = k % RM
                tensor.wait_ge(dve, 4 * k + MD)
                for (c, cw) in chunks[k]:
                    tensor.matmul(
                        ps_cnt.ap()[0:1, 0:cw], ones_b, m_t[jm][:, c : c + cw],
                        start=(n_done == 0), stop=(n_done == cum_ch[-1] - 1),
                    ).then_inc(mm, 1)
                    n_done += 1
            # all sq done: NT sq ops + the hoisted count copy
            tensor.wait_ge(act, NT + 1)
            tensor.matmul(
                ps_sse.ap()[0:1, 0:NT], ones_f, sse_acc[:, :], start=True, stop=True
            ).then_inc(mm, 1)

    nc.finalize()
    return nc


_SSE_OP = None


def _get_sse_op():
    """Register (once) the fused masked-SSE custom DVE op.

    body = ((x*C0 + C1 - s) * (s >= C2))^2, accum_out = per-partition sum.
    With C0=1.75, C1=8, C2=8:  (1.75*(x - t))^2 for late stages, exactly 0
    otherwise (t = (s-8)*4/7, so 1.75*t = s-8).  One 1x DVE instruction per
    tile replaces the xp/m/w0/wm 4-op chain AND the ACT Square pass.
    """
    global _SSE_OP
    if _SSE_OP is not None:
        return _SSE_OP
    from operator import add

    from concourse import dve_ops as _do
    from concourse.dve_spec import C0, C1, C2, Spec, Src0, Src1, lower, sq
    from concourse.dve_uop import DveOpSpec

    def _ref(in0, in1, s0, s1, imm2):
        x = in0.astype(np.float32)
        s = in1.astype(np.float32)
        b = ((x * s0 + s1 - s) * (s >= imm2).astype(np.float32)) ** 2
        b = b.astype(np.float32)
        return b, b.reshape(b.shape[0], -1).sum(axis=-1, keepdims=True)

    spec = Spec(
        body=sq((Src0 * C0 + C1 - Src1) * (Src1 >= C2)),
        accum=add,
        reference=_ref,
    )
    name = "SSE_MASK_ANT"
    shas = {}
    for ver in ("v3", "v4"):
        s = DveOpSpec(name=name, opcode=0, uops=lower(spec, ver=ver), rd1_en=True)
        shas[ver] = s.sha(ver)
    op = _do.DveOp(name, spec, subdim=False, uops_sha=shas)
    if name not in _do._SUB_OPCODE_FOR_NAME:
        _do.OPS.append(op)
        _do.CUSTOM_DVE_SPECS[name] = spec
        _do._SUB_OPCODE_FOR_NAME[name] = max(_do._SUB_OPCODE_FOR_NAME.values()) + 1
    _SSE_OP = op
    return op


def build_v4(shard=SHARD, x_cols=None, s_cols=None):
    """Fused-DVE design: one custom SSE op per x-tile + ACT sigmoid count.

    Per element (x staged bf16, s staged fp8e4 -- both exact enough):
      DVE : sse_acc[:,k] += ((1.75*x + 8 - s) * (s>=8))^2     [1 op/tile, 1x]
      ACT : cnt_acc[:,j] += sigmoid(64*s - 480)  (exact {0,1}) [1 op/s-chunk]
      TE  : ones^T @ acc -> psum[1, nd+ns] (single tiny matmul at the end)
    HBM traffic: 4 MB (x) + 2 MB (s) = 6 MB/core vs 8 MB for the v3 kernel.
    """
    import concourse.bacc as bacc
    from concourse import mybir

    op = _get_sse_op()

    free = shard // P
    if x_cols is None:
        x_cols = [1024, 1536, 2560, 2560, 2560, 2560, 2048, 1536]
    if s_cols is None:
        s_cols = [2048, 5120, 5120, 4096]
    if sum(x_cols) != free:
        nd = 8
        x_cols = [free // nd] * nd
    if sum(s_cols) != free:
        ns = 4
        s_cols = [free // ns] * ns
    assert sum(x_cols) == free and sum(s_cols) == free
    nd, ns = len(x_cols), len(s_cols)
    xp_off = [sum(x_cols[:i]) for i in range(nd + 1)]
    sp_off = [sum(s_cols[:i]) for i in range(ns + 1)]
    # dve tile k needs the s-chunk covering cols [xp_off[k], xp_off[k+1])
    s_for_x = [next(j for j in range(ns) if sp_off[j + 1] >= xp_off[k + 1])
               for k in range(nd)]

    nc = bacc.Bacc("TRN2", target_bir_lowering=False)
    bf16 = mybir.dt.bfloat16
    f32 = mybir.dt.float32
    fp8 = mybir.dt.float8e4
    Act = mybir.ActivationFunctionType

    x_ext = nc.declare_dram_parameter("blast_scores", [shard], bf16, isOutput=False)
    s_ext = nc.declare_dram_parameter("stage_labels", [shard], fp8, isOutput=False)
    out_ext = nc.declare_dram_parameter("out", [nd + ns], f32, isOutput=True)

    x_v = x_ext.ap().rearrange("(p f) -> p f", p=P)
    s_v = s_ext.ap().rearrange("(p f) -> p f", p=P)

    xbuf = nc.alloc_sbuf_tensor("xbuf", [P, free], bf16).ap()
    sbuf = nc.alloc_sbuf_tensor("sbuf", [P, free], fp8).ap()
    scr_d = nc.alloc_sbuf_tensor("scr_d", [P, max(x_cols)], bf16).ap()
    scr_a = nc.alloc_sbuf_tensor("scr_a", [P, max(s_cols)], fp8).ap()
    # acc[:, 0:nd] = per-tile sse partials; acc[:, nd:nd+ns] = count partials
    acc = nc.alloc_sbuf_tensor("acc", [P, nd + ns], f32).ap()
    red = nc.alloc_sbuf_tensor("red", [1, nd + ns], f32).ap()
    sig_bias = nc.alloc_sbuf_tensor("sig_bias", [P, 1], f32).ap()
    ones_f = nc.const_aps.tensor(1.0, (P, 1), f32)

    from contextlib import ExitStack

    with ExitStack() as ctx:
        dx = [ctx.enter_context(nc.semaphore(f"dx{i}")) for i in range(nd)]
        ds = [ctx.enter_context(nc.semaphore(f"ds{j}")) for j in range(ns)]
        dve = ctx.enter_context(nc.semaphore("dve"))
        act = ctx.enter_context(nc.semaphore("act"))
        mm = ctx.enter_context(nc.semaphore("mm"))
        outd = ctx.enter_context(nc.semaphore("outd"))
        bias_rdy = ctx.enter_context(nc.semaphore("bias_rdy"))
        ps = ctx.enter_context(nc.psum_tensor("ps", [1, nd + ns], f32))
        block = ctx.enter_context(nc.Block())

        # interleave x/s chunk issue by byte progress so both streams arrive
        # proportionally (s chunk j before the x tiles that depend on it)
        issue = []
        xi = si = 0
        while xi < nd or si < ns:
            if si < ns and (xi >= nd or si <= s_for_x[min(xi, nd - 1)]):
                issue.append(("s", si)); si += 1
            else:
                issue.append(("x", xi)); xi += 1

        @block.sync
        def _(sync):
            for kind, i in issue:
                if kind == "s":
                    sync.dma_start(
                        out=sbuf[:, sp_off[i] : sp_off[i + 1]],
                        in_=s_v[:, sp_off[i] : sp_off[i + 1]],
                    ).then_inc(ds[i], 16)
                else:
                    sync.dma_start(
                        out=xbuf[:, xp_off[i] : xp_off[i + 1]],
                        in_=x_v[:, xp_off[i] : xp_off[i + 1]],
                    ).then_inc(dx[i], 16)

        @block.vector
        def _(vector):
            for k in range(nd):
                w = x_cols[k]
                vector.wait_ge(ds[s_for_x[k]], 16)
                vector.wait_ge(dx[k], 16)
                vector._custom_dve(
                    op,
                    out=scr_d[:, :w],
                    in0=xbuf[:, xp_off[k] : xp_off[k + 1]],
                    in1=sbuf[:, xp_off[k] : xp_off[k + 1]],
                    s0=1.75,
                    s1=8.0,
                    imm2=8.0,
                    accum_out=acc[:, k : k + 1],
                ).then_inc(dve, 1)

        @block.gpsimd
        def _(gpsimd):
            gpsimd.memset(sig_bias[:, :], -480.0).then_inc(bias_rdy, 1)

        @block.scalar
        def _(scalar):
            # warmup: pull the sigmoid table in while the first DMA lands
            scalar.activation(scr_a[:, 0:1], scr_a[:, 0:1], Act.Sigmoid)
            scalar.wait_ge(bias_rdy, 1)
            for j in range(ns):
                w = s_cols[j]
                scalar.wait_ge(ds[j], 16)
                scalar.activation(
                    scr_a[:, :w],
                    sbuf[:, sp_off[j] : sp_off[j + 1]],
                    Act.Sigmoid,
                    bias=sig_bias[:, :],
                    scale=64.0,
                    accum_out=acc[:, nd + j : nd + j + 1],
                ).then_inc(act, 1)
            scalar.wait_ge(mm, 1)
            scalar.activation(red[0:1, :], ps.ap()[0:1, :], Act.Copy).then_inc(act, 1)
            scalar.wait_ge(act, ns + 1)
            scalar.dma_start(out=out_ext.ap()[:], in_=red[0:1, :]).then_inc(outd, 16)

        @block.tensor
        def _(tensor):
            tensor.wait_ge(dve, nd)
            tensor.wait_ge(act, ns)
            tensor.matmul(
                ps.ap()[0:1, :], ones_f, acc[:, :], start=True, stop=True
            ).then_inc(mm, 1)

    nc.finalize()
    return nc


def build_v5(shard=SHARD, x_cols=None, s_cols=None):
    """v4 + faster ramp and tail.

    - x DMAs on the sync HWDGE ring, s DMAs on the scalar HWDGE ring
      (two rings run concurrently; both streams ramp together)
    - tiny first chunks so DVE/ACT start ~3us earlier
    - no TensorE/PSUM final reduction: the idle sync engine ships the raw
      [128, nd+ns] f32 accumulator tile; host does the final 1.5 KB sum
    - only 3 engine programs (sync/vector/scalar) -> less preamble work
    """
    import concourse.bacc as bacc
    from concourse import mybir

    op = _get_sse_op()

    free = shard // P
    if x_cols is None:
        x_cols = [512, 1024, 2304, 2816, 2816, 2816, 2560, 1536]
    if s_cols is None:
        s_cols = [1024, 3072, 6144, 6144]
    if sum(x_cols) != free:
        x_cols = [free // 8] * 8
    if sum(s_cols) != free:
        s_cols = [free // 4] * 4
    assert sum(x_cols) == free and sum(s_cols) == free
    nd, ns = len(x_cols), len(s_cols)
    xp_off = [sum(x_cols[:i]) for i in range(nd + 1)]
    sp_off = [sum(s_cols[:i]) for i in range(ns + 1)]
    s_for_x = [next(j for j in range(ns) if sp_off[j + 1] >= xp_off[k + 1])
               for k in range(nd)]

    nc = bacc.Bacc("TRN2", target_bir_lowering=False)
    bf16 = mybir.dt.bfloat16
    f32 = mybir.dt.float32
    fp8 = mybir.dt.float8e4
    Act = mybir.ActivationFunctionType

    x_ext = nc.declare_dram_parameter("blast_scores", [shard], bf16, isOutput=False)
    s_ext = nc.declare_dram_parameter("stage_labels", [shard], fp8, isOutput=False)
    out_ext = nc.declare_dram_parameter("out", [P * (nd + ns)], f32, isOutput=True)

    x_v = x_ext.ap().rearrange("(p f) -> p f", p=P)
    s_v = s_ext.ap().rearrange("(p f) -> p f", p=P)
    out_v = out_ext.ap().rearrange("(p f) -> p f", p=P)

    xbuf = nc.alloc_sbuf_tensor("xbuf", [P, free], bf16).ap()
    sbuf = nc.alloc_sbuf_tensor("sbuf", [P, free], fp8).ap()
    scr_d = nc.alloc_sbuf_tensor("scr_d", [P, max(x_cols)], bf16).ap()
    scr_a = nc.alloc_sbuf_tensor("scr_a", [P, max(s_cols)], fp8).ap()
    acc = nc.alloc_sbuf_tensor("acc", [P, nd + ns], f32).ap()
    sig_bias = nc.alloc_sbuf_tensor("sig_bias", [P, 1], f32).ap()

    from contextlib import ExitStack

    with ExitStack() as ctx:
        dx = [ctx.enter_context(nc.semaphore(f"dx{i}")) for i in range(nd)]
        ds = [ctx.enter_context(nc.semaphore(f"ds{j}")) for j in range(ns)]
        dve = ctx.enter_context(nc.semaphore("dve"))
        act = ctx.enter_context(nc.semaphore("act"))
        outd = ctx.enter_context(nc.semaphore("outd"))
        bias_rdy = ctx.enter_context(nc.semaphore("bias_rdy"))
        block = ctx.enter_context(nc.Block())

        @block.sync
        def _(sync):
            for i in range(nd):
                sync.dma_start(
                    out=xbuf[:, xp_off[i] : xp_off[i + 1]],
                    in_=x_v[:, xp_off[i] : xp_off[i + 1]],
                ).then_inc(dx[i], 16)
            sync.wait_ge(dve, nd)
            sync.wait_ge(act, ns)
            sync.dma_start(out=out_v[:, :], in_=acc[:, :]).then_inc(outd, 16)

        @block.vector
        def _(vector):
            vector.memset(sig_bias[:, :], -480.0).then_inc(bias_rdy, 1)
            for k in range(nd):
                w = x_cols[k]
                vector.wait_ge(ds[s_for_x[k]], 16)
                vector.wait_ge(dx[k], 16)
                vector._custom_dve(
                    op,
                    out=scr_d[:, :w],
                    in0=xbuf[:, xp_off[k] : xp_off[k + 1]],
                    in1=sbuf[:, xp_off[k] : xp_off[k + 1]],
                    s0=1.75,
                    s1=8.0,
                    imm2=8.0,
                    accum_out=acc[:, k : k + 1],
                ).then_inc(dve, 1)

        @block.scalar
        def _(scalar):
            for j in range(ns):
                scalar.dma_start(
                    out=sbuf[:, sp_off[j] : sp_off[j + 1]],
                    in_=s_v[:, sp_off[j] : sp_off[j + 1]],
                ).then_inc(ds[j], 16)
            # warmup: pull the sigmoid table in while the first DMA lands
            scalar.activation(scr_a[:, 0:1], scr_a[:, 0:1], Act.Sigmoid)
            scalar.wait_ge(bias_rdy, 1)
            for j in range(ns):
                w = s_cols[j]
                scalar.wait_ge(ds[j], 16)
                scalar.activation(
                    scr_a[:, :w],
                    sbuf[:, sp_off[j] : sp_off[j + 1]],
                    Act.Sigmoid,
                    bias=sig_bias[:, :],
                    scale=64.0,
                    accum_out=acc[:, nd + j : nd + j + 1],
                ).then_inc(act, 1)

    nc.finalize()
    return nc


def build_v6(shard=SHARD, x_cols=None, s_cols=None):
    """v5 + both inputs staged fp8e3 (e3m4): 4 MB/core HBM traffic.

    The custom DVE op runs at 1x regardless of src dtype, so fp8 scores are
    free on the compute side (rel err ~2e-5 vs 7e-7 at bf16 -- gate is 2e-2).
    With DMA (~13 us) far under DVE (~18 us), a single sync-ring stream
    ordered by consumption keeps DVE gapless from the first tile.
    """
    import concourse.bacc as bacc
    from concourse import mybir

    op = _get_sse_op()

    free = shard // P
    if x_cols is None:
        x_cols = [512, 2048, 3072, 3584, 3584, 3584]
    if s_cols is None:
        s_cols = [2048, 4096, 5120, 5120]
    if sum(x_cols) != free:
        x_cols = [free // 8] * 8
    if sum(s_cols) != free:
        s_cols = [free // 4] * 4
    assert sum(x_cols) == free and sum(s_cols) == free
    nd, ns = len(x_cols), len(s_cols)
    xp_off = [sum(x_cols[:i]) for i in range(nd + 1)]
    sp_off = [sum(s_cols[:i]) for i in range(ns + 1)]
    s_for_x = [next(j for j in range(ns) if sp_off[j + 1] >= xp_off[k + 1])
               for k in range(nd)]

    nc = bacc.Bacc("TRN2", target_bir_lowering=False)
    f32 = mybir.dt.float32
    fp8 = mybir.dt.float8e3
    Act = mybir.ActivationFunctionType

    x_ext = nc.declare_dram_parameter("blast_scores", [shard], fp8, isOutput=False)
    s_ext = nc.declare_dram_parameter("stage_labels", [shard], fp8, isOutput=False)
    out_ext = nc.declare_dram_parameter("out", [P * (nd + ns)], f32, isOutput=True)

    x_v = x_ext.ap().rearrange("(p f) -> p f", p=P)
    s_v = s_ext.ap().rearrange("(p f) -> p f", p=P)
    out_v = out_ext.ap().rearrange("(p f) -> p f", p=P)

    xbuf = nc.alloc_sbuf_tensor("xbuf", [P, free], fp8).ap()
    sbuf = nc.alloc_sbuf_tensor("sbuf", [P, free], fp8).ap()
    scr_d = nc.alloc_sbuf_tensor("scr_d", [P, max(x_cols)], mybir.dt.bfloat16).ap()
    scr_a = nc.alloc_sbuf_tensor("scr_a", [P, max(s_cols)], fp8).ap()
    acc = nc.alloc_sbuf_tensor("acc", [P, nd + ns], f32).ap()
    sig_bias = nc.alloc_sbuf_tensor("sig_bias", [P, 1], f32).ap()

    from contextlib import ExitStack

    with ExitStack() as ctx:
        dx = [ctx.enter_context(nc.semaphore(f"dx{i}")) for i in range(nd)]
        ds = [ctx.enter_context(nc.semaphore(f"ds{j}")) for j in range(ns)]
        dve = ctx.enter_context(nc.semaphore("dve"))
        act = ctx.enter_context(nc.semaphore("act"))
        outd = ctx.enter_context(nc.semaphore("outd"))
        bias_rdy = ctx.enter_context(nc.semaphore("bias_rdy"))
        block = ctx.enter_context(nc.Block())

        # single-ring issue order: each s chunk right before the first x
        # chunk that needs it; s0 first so ACT can start asap
        issue = []
        xi = 0
        for j in range(ns):
            issue.append(("s", j))
            while xi < nd and s_for_x[xi] <= j:
                issue.append(("x", xi)); xi += 1

        @block.sync
        def _(sync):
            for kind, i in issue:
                if kind == "s":
                    sync.dma_start(
                        out=sbuf[:, sp_off[i] : sp_off[i + 1]],
                        in_=s_v[:, sp_off[i] : sp_off[i + 1]],
                    ).then_inc(ds[i], 16)
                else:
                    sync.dma_start(
                        out=xbuf[:, xp_off[i] : xp_off[i + 1]],
                        in_=x_v[:, xp_off[i] : xp_off[i + 1]],
                    ).then_inc(dx[i], 16)
            sync.wait_ge(dve, nd)
            sync.wait_ge(act, ns)
            sync.dma_start(out=out_v[:, :], in_=acc[:, :]).then_inc(outd, 16)

        @block.vector
        def _(vector):
            vector.memset(sig_bias[:, :], -480.0).then_inc(bias_rdy, 1)
            for k in range(nd):
                w = x_cols[k]
                vector.wait_ge(ds[s_for_x[k]], 16)
                vector.wait_ge(dx[k], 16)
                vector._custom_dve(
                    op,
                    out=scr_d[:, :w],
                    in0=xbuf[:, xp_off[k] : xp_off[k + 1]],
                    in1=sbuf[:, xp_off[k] : xp_off[k + 1]],
                    s0=1.75,
                    s1=8.0,
                    imm2=8.0,
                    accum_out=acc[:, k : k + 1],
                ).then_inc(dve, 1)

        @block.scalar
        def _(scalar):
            # warmup first: sigmoid tables load while the first DMAs land
            scalar.activation(scr_a[:, 0:1], scr_a[:, 0:1], Act.Sigmoid)
            scalar.wait_ge(bias_rdy, 1)
            for j in range(ns):
                w = s_cols[j]
                scalar.wait_ge(ds[j], 16)
                scalar.activation(
                    scr_a[:, :w],
                    sbuf[:, sp_off[j] : sp_off[j + 1]],
                    Act.Sigmoid,
                    bias=sig_bias[:, :],
                    scale=64.0,
                    accum_out=acc[:, nd + j : nd + j + 1],
                ).then_inc(act, 1)

    nc.finalize()
    return nc


def build_v7(shard=SHARD, x_cols=None, s_cols=None):
    """v6 + three parallel DMA paths so DVE is never starved.

    sync HWDGE ring: x chunks only; gpsimd SWDGE ring: s chunks; scalar:
    pure ACT (tables load at t0).  Both inputs fp8e3 (4 MB/core).
    """
    import concourse.bacc as bacc
    from concourse import mybir

    op = _get_sse_op()

    free = shard // P
    if x_cols is None:
        x_cols = [512, 1536, 2048, 2048, 2560, 2560, 2560, 2560]
    if s_cols is None:
        s_cols = [4096, 6144, 6144]
    if sum(x_cols) != free:
        x_cols = [free // 8] * 8
    if sum(s_cols) != free:
        s_cols = [free // 4] * 4
    assert sum(x_cols) == free and sum(s_cols) == free
    nd, ns = len(x_cols), len(s_cols)
    xp_off = [sum(x_cols[:i]) for i in range(nd + 1)]
    sp_off = [sum(s_cols[:i]) for i in range(ns + 1)]
    s_for_x = [next(j for j in range(ns) if sp_off[j + 1] >= xp_off[k + 1])
               for k in range(nd)]

    nc = bacc.Bacc("TRN2", target_bir_lowering=False)
    f32 = mybir.dt.float32
    fp8 = mybir.dt.float8e3
    Act = mybir.ActivationFunctionType

    x_ext = nc.declare_dram_parameter("blast_scores", [shard], fp8, isOutput=False)
    s_ext = nc.declare_dram_parameter("stage_labels", [shard], fp8, isOutput=False)
    out_ext = nc.declare_dram_parameter("out", [P * (nd + ns)], f32, isOutput=True)

    x_v = x_ext.ap().rearrange("(p f) -> p f", p=P)
    s_v = s_ext.ap().rearrange("(p f) -> p f", p=P)
    out_v = out_ext.ap().rearrange("(p f) -> p f", p=P)

    xbuf = nc.alloc_sbuf_tensor("xbuf", [P, free], fp8).ap()
    sbuf = nc.alloc_sbuf_tensor("sbuf", [P, free], fp8).ap()
    scr_d = nc.alloc_sbuf_tensor("scr_d", [P, max(x_cols)], mybir.dt.bfloat16).ap()
    scr_a = nc.alloc_sbuf_tensor("scr_a", [P, max(s_cols)], fp8).ap()
    acc = nc.alloc_sbuf_tensor("acc", [P, nd + ns], f32).ap()
    sig_bias = nc.alloc_sbuf_tensor("sig_bias", [P, 1], f32).ap()

    from contextlib import ExitStack

    with ExitStack() as ctx:
        dx = [ctx.enter_context(nc.semaphore(f"dx{i}")) for i in range(nd)]
        ds = [ctx.enter_context(nc.semaphore(f"ds{j}")) for j in range(ns)]
        dve = ctx.enter_context(nc.semaphore("dve"))
        act = ctx.enter_context(nc.semaphore("act"))
        outd = ctx.enter_context(nc.semaphore("outd"))
        bias_rdy = ctx.enter_context(nc.semaphore("bias_rdy"))
        block = ctx.enter_context(nc.Block())

        @block.sync
        def _(sync):
            for i in range(nd):
                sync.dma_start(
                    out=xbuf[:, xp_off[i] : xp_off[i + 1]],
                    in_=x_v[:, xp_off[i] : xp_off[i + 1]],
                ).then_inc(dx[i], 16)
            sync.wait_ge(dve, nd)
            sync.wait_ge(act, ns)
            sync.dma_start(out=out_v[:, :], in_=acc[:, :]).then_inc(outd, 16)

        @block.gpsimd
        def _(gpsimd):
            gpsimd.memset(sig_bias[:, :], -480.0).then_inc(bias_rdy, 1)
            for j in range(ns):
                gpsimd.dma_start(
                    out=sbuf[:, sp_off[j] : sp_off[j + 1]],
                    in_=s_v[:, sp_off[j] : sp_off[j + 1]],
                ).then_inc(ds[j], 16)

        @block.vector
        def _(vector):
            for k in range(nd):
                w = x_cols[k]
                vector.wait_ge(ds[s_for_x[k]], 16)
                vector.wait_ge(dx[k], 16)
                vector._custom_dve(
                    op,
                    out=scr_d[:, :w],
                    in0=xbuf[:, xp_off[k] : xp_off[k + 1]],
                    in1=sbuf[:, xp_off[k] : xp_off[k + 1]],
                    s0=1.75,
                    s1=8.0,
                    imm2=8.0,
                    accum_out=acc[:, k : k + 1],
                ).then_inc(dve, 1)

        @block.scalar
        def _(scalar):
            # warmup first: sigmoid tables load while the first DMAs land
            scalar.activation(scr_a[:, 0:1], scr_a[:, 0:1], Act.Sigmoid)
            scalar.wait_ge(bias_rdy, 1)
            for j in range(ns):
                w = s_cols[j]
                scalar.wait_ge(ds[j], 16)
                scalar.activation(
                    scr_a[:, :w],
                    sbuf[:, sp_off[j] : sp_off[j + 1]],
                    Act.Sigmoid,
                    bias=sig_bias[:, :],
                    scale=64.0,
                    accum_out=acc[:, nd + j : nd + j + 1],
                ).then_inc(act, 1)

    nc.finalize()
    return nc


def _to_bf16(a):
    import ml_dtypes

    return np.ascontiguousarray(a.astype(ml_dtypes.bfloat16))


def _to_fp8(a):
    import ml_dtypes

    return np.ascontiguousarray(a.astype(np.float32).astype(ml_dtypes.float8_e4m3fn))


def _to_fp8e3(a):
    import ml_dtypes

    return np.ascontiguousarray(a.astype(np.float32).astype(ml_dtypes.float8_e3m4))


def run(x, s, variant="v3", **spmd_kwargs):
    """Shard, run on 8 cores, host-reduce. Returns (loss, BassKernelResults)."""
    from concourse.bass_utils import run_bass_kernel_spmd

    if variant not in _NC_CACHE:
        if variant == "raw":
            _NC_CACHE[variant] = build_raw()
        elif variant == "v3":
            _NC_CACHE[variant] = build_v3()
        elif variant == "v4":
            _NC_CACHE[variant] = build_v4()
        elif variant == "v5":
            _NC_CACHE[variant] = build_v5()
        elif variant == "v6":
            _NC_CACHE[variant] = build_v6()
        elif variant == "v7":
            _NC_CACHE[variant] = build_v7()
        else:
            raise ValueError(variant)
    nc = _NC_CACHE[variant]

    if variant == "raw":
        xs, ss = x, s
    elif variant in ("v6", "v7"):
        xs, ss = _to_fp8e3(x), _to_fp8e3(s)
    elif variant in ("v4", "v5"):
        xs, ss = _to_bf16(x), _to_fp8(s)
    else:
        xs, ss = _to_bf16(x), _to_bf16(s)

    in_maps = [
        {
            "blast_scores": xs[i * SHARD : (i + 1) * SHARD],
            "stage_labels": ss[i * SHARD : (i + 1) * SHARD],
        }
        for i in range(N_CORES)
    ]
    res = run_bass_kernel_spmd(nc, in_maps, core_ids=list(range(N_CORES)), **spmd_kwargs)

    cnt = 0.0
    sse = 0.0
    for r in res.results:
        o = r["out"].astype(np.float64)
        if variant == "raw":
            o = o.reshape(2, -1)
            cnt += o[0].sum()
            sse += o[1].sum()
        elif variant == "v4":
            sse += o[:8].sum() / 3.0625  # undo the 1.75^2 prescale
            cnt += o[8:].sum()
        elif variant == "v5":
            o = o.reshape(P, 12)
            sse += o[:, :8].sum() / 3.0625
            cnt += o[:, 8:].sum()
        elif variant == "v6":
            o = o.reshape(P, 10)
            sse += o[:, :6].sum() / 3.0625
            cnt += o[:, 6:].sum()
        elif variant == "v7":
            o = o.reshape(P, 11)
            sse += o[:, :8].sum() / 3.0625
            cnt += o[:, 8:].sum()
        else:
            cnt += o[:512].sum()
            sse += o[512:].sum()
    val = sse / max(cnt, 1.0) if cnt > 0 else 0.0
    return np.asarray(val, dtype=np.float32), res


def kernel(**inputs):
    x = np.ascontiguousarray(np.asarray(inputs["blast_scores"], dtype=np.float32))
    s = np.ascontiguousarray(np.asarray(inputs["stage_labels"], dtype=np.int32))
    assert x.shape == (B,) and s.shape == (B,)
    return run(x, s)[0]



# revision 20
# speedup vs baseline: 1.0190x; 1.0190x over previous
"""Bass/Trainium2 kernel for nn_BlastocystAuxLoss.

Computes a masked MSE over B=16,777,216 elements:
    late stages are labels 8..15; target[s] = (s-8) * 4/7 for late stages;
    loss = sum_{s>=8} (x - target)^2 / count(s>=8)   (0.0 if count == 0)

Strategy: trivially data-parallel over 8 NeuronCores. Inputs are staged to
the device as bf16 (labels 0..15 are exact in bf16; scores were already
rounded to bf16 inside the original f32 kernel's DVE ops, so accuracy is
unchanged at ~3e-6 rel err) which halves HBM traffic to 8 MB per core.
Each core computes per-partition partial {count, sse} on-chip and ships a
tiny [520] f32 partials row; the final reduction (sum + divide) happens on
host in f64. No collectives needed.

Engine split (see build_v3 for the measured perf-mode rules it encodes):
    DVE: xp = 1.75*x+8 [4x], m = (s>=8) [4x], w0 = xp-s [2x], wm = w0*m [2x]
    ACT: sq = Square(4/7*wm) with free accum -> sse   (exact 0 when m=0)
    TE : ones^T @ m in 512-col chunks -> PSUM -> count
"""

from contextlib import ExitStack

import numpy as np

B = 16777216
N_CORES = 8
SHARD = B // N_CORES  # 2,097,152
P = 128

_NC_CACHE = {}


def build(shard=SHARD, n_tiles=8):
    """Build the single-core Bass program (same SPMD program for all cores)."""
    import concourse.bacc as bacc
    import concourse.tile as tile
    from concourse import mybir

    free = shard // P
    fd = free // n_tiles
    assert fd * n_tiles * P == shard

    nc = bacc.Bacc("TRN2", target_bir_lowering=False)
    x_ext = nc.declare_dram_parameter(
        "blast_scores", [shard], mybir.dt.float32, isOutput=False
    )
    s_ext = nc.declare_dram_parameter(
        "stage_labels", [shard], mybir.dt.int32, isOutput=False
    )
    out_ext = nc.declare_dram_parameter("out", [P, 2], mybir.dt.float32, isOutput=True)

    x_v = x_ext.ap().rearrange("(p f) -> p f", p=P)
    s_v = s_ext.ap().rearrange("(p f) -> p f", p=P)

    c47 = 4.0 / 7.0  # target step; folded into the Square's input scale
    c74 = 7.0 / 4.0  # x prescale so z = 7/4*(x - t) uses integer-exact v

    f32 = mybir.dt.float32
    bf16 = mybir.dt.bfloat16
    Alu = mybir.AluOpType
    Act = mybir.ActivationFunctionType

    with tile.TileContext(nc) as tc:
        with (
            tc.tile_pool(name="io", bufs=4) as io_pool,
            tc.tile_pool(name="mid", bufs=3) as mid_pool,
            tc.tile_pool(name="acc", bufs=1) as acc_pool,
        ):
            cnt_acc = acc_pool.tile([P, n_tiles], f32)
            sse_acc = acc_pool.tile([P, n_tiles], f32)
            red = acc_pool.tile([P, 2], f32)
            # bias for the sigmoid step mask: m = sigmoid(64*s - 480)
            sig_bias = acc_pool.tile([P, 1], f32)
            nc.gpsimd.memset(sig_bias[:], -480.0)

            for k in range(n_tiles):
                x_t = io_pool.tile([P, fd], f32, tag="x")
                s_t = io_pool.tile([P, fd], mybir.dt.int32, tag="s")
                nc.sync.dma_start(out=x_t[:], in_=x_v[:, k * fd : (k + 1) * fd])
                nc.sync.dma_start(out=s_t[:], in_=s_v[:, k * fd : (k + 1) * fd])

                m = mid_pool.tile([P, fd], bf16, tag="m")
                v = mid_pool.tile([P, fd], bf16, tag="v")
                z = mid_pool.tile([P, fd], bf16, tag="z")
                zm = mid_pool.tile([P, fd], bf16, tag="zm")
                sq = mid_pool.tile([P, fd], bf16, tag="sq")

                # ACT: step mask m = sigmoid(64*(s - 7.5)) in {0,1} exactly
                # (saturated at +-32); accumulate count for free
                nc.scalar.activation(
                    m[:], s_t[:], Act.Sigmoid, bias=sig_bias[:], scale=64.0,
                    accum_out=cnt_acc[:, k : k + 1],
                )
                # DVE: v = max(s-8, 0)
                nc.vector.tensor_scalar(v[:], s_t[:], 8, 0, Alu.subtract, Alu.max)
                # DVE: z = 7/4*x - v  (== 7/4*(x - target) since v = 7/4*t)
                nc.vector.scalar_tensor_tensor(
                    z[:], x_t[:], c74, v[:], Alu.mult, Alu.subtract
                )
                nc.vector.tensor_tensor(zm[:], z[:], m[:], Alu.mult)
                # ACT: sse += (4/7 * zm)^2 over masked elements
                nc.scalar.activation(
                    sq[:], zm[:], Act.Square, scale=c47,
                    accum_out=sse_acc[:, k : k + 1],
                )

            nc.vector.reduce_sum(red[:, 0:1], cnt_acc[:], axis=mybir.AxisListType.X)
            nc.vector.reduce_sum(red[:, 1:2], sse_acc[:], axis=mybir.AxisListType.X)
            nc.sync.dma_start(out=out_ext.ap()[:, :], in_=red[:])

    nc.finalize()
    return nc


def build_raw(shard=2097152, sizes=None, ring=6):
    """Hand-scheduled raw-Bass builder (no TileContext).

    - per-slot DMA semaphores (multi-queue completions are unordered);
      slot reuse (tile k vs k+R) is ordered by issue-side consumer waits
    - ring of 6 slots so DMA issue never gates on compute and the input
      stream stays bandwidth-bound end to end
    - tile sizes taper at the end so the last tile's compute lag after
      the final (bandwidth-bound) DMA is minimal
    - final reduction via a TensorEngine ones-matmul (cross-partition sum
      -> PSUM [1, 2*NT]) so the output DMA is one small descriptor instead
      of 128 8-byte ones
    """
    import concourse.bacc as bacc
    from concourse import mybir

    free = shard // P
    if sizes is None:
        sizes = [2048] * 7 + [1536, 512]
        if sum(sizes) != free:  # non-default shard (tests)
            fd = free // 8
            sizes = [fd] * 8
    assert sum(sizes) == free
    fd = max(sizes)
    NT = len(sizes)
    offs = [sum(sizes[:i]) for i in range(NT)]
    R = min(ring, NT)

    nc = bacc.Bacc("TRN2", target_bir_lowering=False)
    x_ext = nc.declare_dram_parameter(
        "blast_scores", [shard], mybir.dt.float32, isOutput=False
    )
    s_ext = nc.declare_dram_parameter(
        "stage_labels", [shard], mybir.dt.int32, isOutput=False
    )
    out_ext = nc.declare_dram_parameter("out", [2 * NT], mybir.dt.float32, isOutput=True)

    x_v = x_ext.ap().rearrange("(p f) -> p f", p=P)
    s_v = s_ext.ap().rearrange("(p f) -> p f", p=P)

    c47 = 4.0 / 7.0
    c74 = 7.0 / 4.0

    f32 = mybir.dt.float32
    i32 = mybir.dt.int32
    bf16 = mybir.dt.bfloat16
    Alu = mybir.AluOpType
    Act = mybir.ActivationFunctionType

    x_t = [nc.alloc_sbuf_tensor(f"x{i}", [P, fd], f32).ap() for i in range(R)]
    s_t = [nc.alloc_sbuf_tensor(f"s{i}", [P, fd], i32).ap() for i in range(R)]
    m_t = [nc.alloc_sbuf_tensor(f"m{i}", [P, fd], bf16).ap() for i in range(R)]
    v_t = [nc.alloc_sbuf_tensor(f"v{i}", [P, fd], bf16).ap() for i in range(2)]
    z_t = [nc.alloc_sbuf_tensor(f"z{i}", [P, fd], bf16).ap() for i in range(2)]
    zm_t = [nc.alloc_sbuf_tensor(f"zm{i}", [P, fd], bf16).ap() for i in range(R)]
    sq_t = nc.alloc_sbuf_tensor("sq", [P, fd], bf16).ap()
    # acc[:, k] = per-partition count of tile k; acc[:, NT+k] = partial sse
    acc = nc.alloc_sbuf_tensor("acc", [P, 2 * NT], f32).ap()
    red1 = nc.alloc_sbuf_tensor("red1", [1, 2 * NT], f32).ap()
    sig_bias = nc.alloc_sbuf_tensor("sig_bias", [P, 1], f32).ap()
    ones = nc.const_aps.tensor(1.0, (P, 1), f32)

    with ExitStack() as ctx:
        dma_x = [ctx.enter_context(nc.semaphore(f"dma_x{i}")) for i in range(R)]
        dma_s = [ctx.enter_context(nc.semaphore(f"dma_s{i}")) for i in range(R)]
        dve = ctx.enter_context(nc.semaphore("dve"))
        act = ctx.enter_context(nc.semaphore("act"))
        mm = ctx.enter_context(nc.semaphore("mm"))
        outd = ctx.enter_context(nc.semaphore("outd"))
        bias_rdy = ctx.enter_context(nc.semaphore("bias_rdy"))
        psum = ctx.enter_context(nc.psum_tensor("ps", [1, 2 * NT], f32))
        block = ctx.enter_context(nc.Block())

        # Semaphore increment ledger:
        #   DVE: 3 per tile (v, z, zm)            -> 3*NT total
        #   ACT: 2 per tile (m, sq) + final copy  -> 2*NT + 1 total
        #   DMA slot sems: +16 per transfer into that slot

        @block.sync
        def _(sync):
            for k in range(NT):
                i = k % R
                w = sizes[k]
                if k >= R:
                    # x slot free when z(k-R) done; s slot free when
                    # v(k-R) (implied by z) and m(k-R) done
                    sync.wait_ge(dve, 3 * (k - R) + 2)
                    sync.wait_ge(act, 2 * (k - R) + 1)
                sync.dma_start(
                    out=s_t[i][:, :w], in_=s_v[:, offs[k] : offs[k] + w]
                ).then_inc(dma_s[i], 16)
                sync.dma_start(
                    out=x_t[i][:, :w], in_=x_v[:, offs[k] : offs[k] + w]
                ).then_inc(dma_x[i], 16)
            sync.wait_ge(act, 2 * NT + 1)  # final ScE copy done
            sync.dma_start(out=out_ext.ap()[:], in_=red1[0:1, :]).then_inc(outd, 16)
            if not skip_out_wait:
                sync.wait_ge(outd, 16)

        @block.vector
        def _(vector):
            vector.memset(sig_bias[:, :], -480.0).then_inc(bias_rdy, 1)
            for k in range(NT):
                i = k % R
                w = sizes[k]
                rnd = 16 * (k // R + 1)
                # v = max(s-8, 0)
                vector.wait_ge(dma_s[i], rnd)
                vector.tensor_scalar(
                    v_t[k % 2][:, :w], s_t[i][:, :w], 8, 0, Alu.subtract, Alu.max
                ).then_inc(dve, 1)
                # z = 7/4*x - v
                vector.wait_ge(dma_x[i], rnd)
                vector.wait_ge(dve, 3 * k + 1)  # v(k) drained
                vector.scalar_tensor_tensor(
                    z_t[k % 2][:, :w], x_t[i][:, :w], c74, v_t[k % 2][:, :w],
                    Alu.mult, Alu.subtract,
                ).then_inc(dve, 1)
                # zm = z * m   (m(k) ready when act >= 2k+1)
                vector.wait_ge(act, 2 * k + 1)
                vector.wait_ge(dve, 3 * k + 2)  # z(k) drained
                vector.tensor_tensor(
                    zm_t[i][:, :w], z_t[k % 2][:, :w], m_t[i][:, :w], Alu.mult
                ).then_inc(dve, 1)

        @block.scalar
        def _(scalar):
            scalar.wait_ge(bias_rdy, 1)
            for k in range(NT):
                i = k % R
                w = sizes[k]
                rnd = 16 * (k // R + 1)
                # m = sigmoid(64*s - 480) in {0,1}; count accumulates free
                scalar.wait_ge(dma_s[i], rnd)
                if k >= R:
                    # m slot free when zm(k-R) done
                    scalar.wait_ge(dve, 3 * (k - R) + 3)
                scalar.activation(
                    m_t[i][:, :w], s_t[i][:, :w], Act.Sigmoid,
                    bias=sig_bias[:, :], scale=64.0,
                    accum_out=acc[:, k : k + 1],
                ).then_inc(act, 1)
                # sq = Square(zm * 4/7); sse accum; zm(k): dve >= 3k+3
                scalar.wait_ge(dve, 3 * k + 3)
                scalar.activation(
                    sq_t[:, :w], zm_t[i][:, :w], Act.Square, scale=c47,
                    accum_out=acc[:, NT + k : NT + k + 1],
                ).then_inc(act, 1)
            # after the matmul: PSUM -> SBUF single-partition copy, then
            # ship the 2*NT partials out (single 8*2*NT-byte descriptor);
            # issuing here avoids a cross-engine hop before the final DMA
            scalar.wait_ge(mm, 1)
            scalar.activation(red1[0:1, :], psum.ap()[0:1, :], Act.Copy).then_inc(
                act, 1
            )

        @block.tensor
        def _(tensor):
            # cross-partition reduction: ones.T @ acc -> [1, 2*NT]
            tensor.wait_ge(act, 2 * NT)
            tensor.wait_ge(dve, 3 * NT)
            tensor.matmul(psum.ap()[0:1, :], ones, acc[:, :]).then_inc(mm, 1)

    nc.finalize()
    return nc


def build_v3(shard=SHARD, sizes=None, ring=4):
    """bf16-staged pipeline, fast-mode ops only, x-first tile order.

    Mode rules this is built around (all hardware-measured):
      - DVE tensor_scalar (incl. is_ge): 4x mode; tensor_tensor: 2x;
        scalar_tensor_tensor / accum_out on DVE: 1x (avoided)
      - ACT: 1 elem/cycle/lane, accum_out free -> owns Square + sse
      - TensorE: ones-matmul count accumulation into PSUM (pays ~1.5us of
        DVE SBUF-port contention; cheaper than any accumulating DVE op)
      - output DMAs issued from the Scalar engine (HWDGE); no completion
        wait needed -- the runtime drains DMA queues at NEFF end

    Per element (s = label, x = score, both staged bf16 from host):
      DVE: xp = 1.75*x + 8        [ts 4x]   (x arrives first, so xp leads)
      DVE: m  = (s >= 8)          [ts 4x]
      DVE: w0 = xp - s            [tt 2x]   (masked: 7/4*(x - target))
      DVE: wm = w0 * m            [tt 2x]   (exactly 0 when unmasked)
      ACT: sq = Square(4/7 * wm)  accum -> sse partials
      TE : ones^T @ m chunks -> PSUM[1,512] -> count
    """
    import concourse.bacc as bacc
    from concourse import mybir

    free = shard // P
    if sizes is None:
        sizes = [1024, 1536, 2048, 2560, 3072, 3072, 2560, 512]
        if sum(sizes) != free:  # non-default shard (tests)
            fd = free // 8
            sizes = [fd] * 8
    assert sum(sizes) == free
    fd = max(sizes)
    NT = len(sizes)
    offs = [sum(sizes[:i]) for i in range(NT)]
    R = min(ring, NT)
    CW = 512
    chunks = [
        [(c, min(CW, sizes[k] - c)) for c in range(0, sizes[k], CW)]
        for k in range(NT)
    ]
    cum_ch = [0]
    for k in range(NT):
        cum_ch.append(cum_ch[-1] + len(chunks[k]))
    n_mm = cum_ch[-1] + 1  # + final sse reduction

    nc = bacc.Bacc("TRN2", target_bir_lowering=False)
    bf16 = mybir.dt.bfloat16
    f32 = mybir.dt.float32
    Alu = mybir.AluOpType
    Act = mybir.ActivationFunctionType

    x_ext = nc.declare_dram_parameter("blast_scores", [shard], bf16, isOutput=False)
    s_ext = nc.declare_dram_parameter("stage_labels", [shard], bf16, isOutput=False)
    out_ext = nc.declare_dram_parameter("out", [CW + NT], f32, isOutput=True)

    x_v = x_ext.ap().rearrange("(p f) -> p f", p=P)
    s_v = s_ext.ap().rearrange("(p f) -> p f", p=P)

    x_t = [nc.alloc_sbuf_tensor(f"x{i}", [P, fd], bf16).ap() for i in range(R)]
    s_t = [nc.alloc_sbuf_tensor(f"s{i}", [P, fd], bf16).ap() for i in range(R)]
    xp_t = [nc.alloc_sbuf_tensor(f"xp{i}", [P, fd], bf16).ap() for i in range(2)]
    RM = 3
    m_t = [nc.alloc_sbuf_tensor(f"m{i}", [P, fd], bf16).ap() for i in range(RM)]
    w0_t = [nc.alloc_sbuf_tensor(f"w0{i}", [P, fd], bf16).ap() for i in range(2)]
    RW = 3
    wm_t = [nc.alloc_sbuf_tensor(f"wm{i}", [P, fd], bf16).ap() for i in range(RW)]
    sq_t = nc.alloc_sbuf_tensor("sq", [P, fd], bf16).ap()
    sse_acc = nc.alloc_sbuf_tensor("sse_acc", [P, NT], f32).ap()
    red1 = nc.alloc_sbuf_tensor("red1", [1, CW + NT], f32).ap()
    ones_b = nc.const_aps.tensor(1.0, (P, 1), bf16)
    ones_f = nc.const_aps.tensor(1.0, (P, 1), f32)

    # DVE op retirement offsets within tile k (4 ops/tile):
    XPD, MD, W0D, WMD = 1, 2, 3, 4

    with ExitStack() as ctx:
        dma_x = [ctx.enter_context(nc.semaphore(f"dma_x{i}")) for i in range(R)]
        dma_s = [ctx.enter_context(nc.semaphore(f"dma_s{i}")) for i in range(R)]
        dve = ctx.enter_context(nc.semaphore("dve"))
        act = ctx.enter_context(nc.semaphore("act"))
        mm = ctx.enter_context(nc.semaphore("mm"))
        outd = ctx.enter_context(nc.semaphore("outd"))
        ps_cnt = ctx.enter_context(nc.psum_tensor("pscnt", [1, CW], f32))
        ps_sse = ctx.enter_context(nc.psum_tensor("pssse", [1, NT], f32))
        block = ctx.enter_context(nc.Block())

        @block.sync
        def _(sync):
            for k in range(NT):
                i = k % R
                w = sizes[k]
                if k >= R:
                    # x slot freed by xp(k-R); s slot by w0(k-R)
                    sync.wait_ge(dve, 4 * (k - R) + W0D)
                sync.dma_start(
                    out=x_t[i][:, :w], in_=x_v[:, offs[k] : offs[k] + w]
                ).then_inc(dma_x[i], 16)
                sync.dma_start(
                    out=s_t[i][:, :w], in_=s_v[:, offs[k] : offs[k] + w]
                ).then_inc(dma_s[i], 16)

        @block.vector
        def _(vector):
            for k in range(NT):
                i = k % R
                w = sizes[k]
                rnd = 16 * (k // R + 1)
                jm = k % RM
                jw = k % RW
                # xp = 1.75*x + 8  [4x]
                vector.wait_ge(dma_x[i], rnd)
                vector.tensor_scalar(
                    xp_t[k % 2][:, :w], x_t[i][:, :w], 1.75, 8.0, Alu.mult, Alu.add
                ).then_inc(dve, 1)
                # m = (s >= 8)  [4x]
                vector.wait_ge(dma_s[i], rnd)
                if k >= RM:
                    # m slot reused: freed when TE finishes tile k-RM
                    vector.wait_ge(mm, cum_ch[k - RM + 1])
                vector.tensor_scalar(
                    m_t[jm][:, :w], s_t[i][:, :w], 8.0, 0.0, Alu.is_ge, Alu.add
                ).then_inc(dve, 1)
                # w0 = xp - s  [2x]
                vector.tensor_tensor(
                    w0_t[k % 2][:, :w], xp_t[k % 2][:, :w], s_t[i][:, :w],
                    Alu.subtract,
                ).then_inc(dve, 1)
                # wm = w0 * m  [2x]
                if k >= RW:
                    # wm slot reused: freed by ACT sq(k-RW)
                    vector.wait_ge(act, k - RW + 1)
                vector.tensor_tensor(
                    wm_t[jw][:, :w], w0_t[k % 2][:, :w], m_t[jm][:, :w], Alu.mult
                ).then_inc(dve, 1)

        @block.scalar
        def _(scalar):
            c47 = 4.0 / 7.0
            acts = 0
            for k in range(NT):
                w = sizes[k]
                jw = k % RW
                if k == NT - 1:
                    # count matmuls all retire with m(NT-1); ship the count
                    # half of the output while the last tile still computes
                    scalar.wait_ge(mm, n_mm - 1)
                    scalar.activation(
                        red1[0:1, 0:CW], ps_cnt.ap()[0:1, :], Act.Copy
                    ).then_inc(act, 1)
                    acts += 1
                    # the sequencer runs ahead of the ACT datapath: wait for
                    # the copy to land before the DMA reads red1
                    scalar.wait_ge(act, acts)
                    scalar.dma_start(
                        out=out_ext.ap()[0:CW], in_=red1[0:1, 0:CW]
                    ).then_inc(outd, 16)
                scalar.wait_ge(dve, 4 * k + WMD)
                scalar.activation(
                    sq_t[:, :w], wm_t[jw][:, :w], Act.Square, scale=c47,
                    accum_out=sse_acc[:, k : k + 1],
                ).then_inc(act, 1)
                acts += 1
            scalar.wait_ge(mm, n_mm)
            scalar.activation(
                red1[0:1, CW : CW + NT], ps_sse.ap()[0:1, :], Act.Copy
            ).then_inc(act, 1)
            scalar.wait_ge(act, NT + 2)
            # runtime drains DMA queues at NEFF end; no completion wait
            scalar.dma_start(
                out=out_ext.ap()[CW : CW + NT], in_=red1[0:1, CW : CW + NT]
            ).then_inc(outd, 16)

        @block.tensor
        def _(tensor):
            n_done = 0
            for k in range(NT):
                jm = k % RM
                tensor.wait_ge(dve, 4 * k + MD)
                for (c, cw) in chunks[k]:
                    tensor.matmul(
                        ps_cnt.ap()[0:1, 0:cw], ones_b, m_t[jm][:, c : c + cw],
                        start=(n_done == 0), stop=(n_done == cum_ch[-1] - 1),
                    ).then_inc(mm, 1)
                    n_done += 1
            # all sq done: NT sq ops + the hoisted count copy
            tensor.wait_ge(act, NT + 1)
            tensor.matmul(
                ps_sse.ap()[0:1, 0:NT], ones_f, sse_acc[:, :], start=True, stop=True
            ).then_inc(mm, 1)

    nc.finalize()
    return nc


_SSE_OP = None


def _get_sse_op():
    """Register (once) the fused masked-SSE custom DVE op.

    body = ((x*C0 + C1 - s) * (s >= C2))^2, accum_out = per-partition sum.
    With C0=1.75, C1=8, C2=8:  (1.75*(x - t))^2 for late stages, exactly 0
    otherwise (t = (s-8)*4/7, so 1.75*t = s-8).  One 1x DVE instruction per
    tile replaces the xp/m/w0/wm 4-op chain AND the ACT Square pass.
    """
    global _SSE_OP
    if _SSE_OP is not None:
        return _SSE_OP
    from operator import add

    from concourse import dve_ops as _do
    from concourse.dve_spec import C0, C1, C2, Spec, Src0, Src1, lower, sq
    from concourse.dve_uop import DveOpSpec

    def _ref(in0, in1, s0, s1, imm2):
        x = in0.astype(np.float32)
        s = in1.astype(np.float32)
        b = ((x * s0 + s1 - s) * (s >= imm2).astype(np.float32)) ** 2
        b = b.astype(np.float32)
        return b, b.reshape(b.shape[0], -1).sum(axis=-1, keepdims=True)

    spec = Spec(
        body=sq((Src0 * C0 + C1 - Src1) * (Src1 >= C2)),
        accum=add,
        reference=_ref,
    )
    name = "SSE_MASK_ANT"
    shas = {}
    for ver in ("v3", "v4"):
        s = DveOpSpec(name=name, opcode=0, uops=lower(spec, ver=ver), rd1_en=True)
        shas[ver] = s.sha(ver)
    op = _do.DveOp(name, spec, subdim=False, uops_sha=shas)
    if name not in _do._SUB_OPCODE_FOR_NAME:
        _do.OPS.append(op)
        _do.CUSTOM_DVE_SPECS[name] = spec
        _do._SUB_OPCODE_FOR_NAME[name] = max(_do._SUB_OPCODE_FOR_NAME.values()) + 1
    _SSE_OP = op
    return op


def build_v4(shard=SHARD, x_cols=None, s_cols=None):
    """Fused-DVE design: one custom SSE op per x-tile + ACT sigmoid count.

    Per element (x staged bf16, s staged fp8e4 -- both exact enough):
      DVE : sse_acc[:,k] += ((1.75*x + 8 - s) * (s>=8))^2     [1 op/tile, 1x]
      ACT : cnt_acc[:,j] += sigmoid(64*s - 480)  (exact {0,1}) [1 op/s-chunk]
      TE  : ones^T @ acc -> psum[1, nd+ns] (single tiny matmul at the end)
    HBM traffic: 4 MB (x) + 2 MB (s) = 6 MB/core vs 8 MB for the v3 kernel.
    """
    import concourse.bacc as bacc
    from concourse import mybir

    op = _get_sse_op()

    free = shard // P
    if x_cols is None:
        x_cols = [1024, 1536, 2560, 2560, 2560, 2560, 2048, 1536]
    if s_cols is None:
        s_cols = [2048, 5120, 5120, 4096]
    if sum(x_cols) != free:
        nd = 8
        x_cols = [free // nd] * nd
    if sum(s_cols) != free:
        ns = 4
        s_cols = [free // ns] * ns
    assert sum(x_cols) == free and sum(s_cols) == free
    nd, ns = len(x_cols), len(s_cols)
    xp_off = [sum(x_cols[:i]) for i in range(nd + 1)]
    sp_off = [sum(s_cols[:i]) for i in range(ns + 1)]
    # dve tile k needs the s-chunk covering cols [xp_off[k], xp_off[k+1])
    s_for_x = [next(j for j in range(ns) if sp_off[j + 1] >= xp_off[k + 1])
               for k in range(nd)]

    nc = bacc.Bacc("TRN2", target_bir_lowering=False)
    bf16 = mybir.dt.bfloat16
    f32 = mybir.dt.float32
    fp8 = mybir.dt.float8e4
    Act = mybir.ActivationFunctionType

    x_ext = nc.declare_dram_parameter("blast_scores", [shard], bf16, isOutput=False)
    s_ext = nc.declare_dram_parameter("stage_labels", [shard], fp8, isOutput=False)
    out_ext = nc.declare_dram_parameter("out", [nd + ns], f32, isOutput=True)

    x_v = x_ext.ap().rearrange("(p f) -> p f", p=P)
    s_v = s_ext.ap().rearrange("(p f) -> p f", p=P)

    xbuf = nc.alloc_sbuf_tensor("xbuf", [P, free], bf16).ap()
    sbuf = nc.alloc_sbuf_tensor("sbuf", [P, free], fp8).ap()
    scr_d = nc.alloc_sbuf_tensor("scr_d", [P, max(x_cols)], bf16).ap()
    scr_a = nc.alloc_sbuf_tensor("scr_a", [P, max(s_cols)], fp8).ap()
    # acc[:, 0:nd] = per-tile sse partials; acc[:, nd:nd+ns] = count partials
    acc = nc.alloc_sbuf_tensor("acc", [P, nd + ns], f32).ap()
    red = nc.alloc_sbuf_tensor("red", [1, nd + ns], f32).ap()
    sig_bias = nc.alloc_sbuf_tensor("sig_bias", [P, 1], f32).ap()
    ones_f = nc.const_aps.tensor(1.0, (P, 1), f32)

    from contextlib import ExitStack

    with ExitStack() as ctx:
        dx = [ctx.enter_context(nc.semaphore(f"dx{i}")) for i in range(nd)]
        ds = [ctx.enter_context(nc.semaphore(f"ds{j}")) for j in range(ns)]
        dve = ctx.enter_context(nc.semaphore("dve"))
        act = ctx.enter_context(nc.semaphore("act"))
        mm = ctx.enter_context(nc.semaphore("mm"))
        outd = ctx.enter_context(nc.semaphore("outd"))
        bias_rdy = ctx.enter_context(nc.semaphore("bias_rdy"))
        ps = ctx.enter_context(nc.psum_tensor("ps", [1, nd + ns], f32))
        block = ctx.enter_context(nc.Block())

        # interleave x/s chunk issue by byte progress so both streams arrive
        # proportionally (s chunk j before the x tiles that depend on it)
        issue = []
        xi = si = 0
        while xi < nd or si < ns:
            if si < ns and (xi >= nd or si <= s_for_x[min(xi, nd - 1)]):
                issue.append(("s", si)); si += 1
            else:
                issue.append(("x", xi)); xi += 1

        @block.sync
        def _(sync):
            for kind, i in issue:
                if kind == "s":
                    sync.dma_start(
                        out=sbuf[:, sp_off[i] : sp_off[i + 1]],
                        in_=s_v[:, sp_off[i] : sp_off[i + 1]],
                    ).then_inc(ds[i], 16)
                else:
                    sync.dma_start(
                        out=xbuf[:, xp_off[i] : xp_off[i + 1]],
                        in_=x_v[:, xp_off[i] : xp_off[i + 1]],
                    ).then_inc(dx[i], 16)

        @block.vector
        def _(vector):
            for k in range(nd):
                w = x_cols[k]
                vector.wait_ge(ds[s_for_x[k]], 16)
                vector.wait_ge(dx[k], 16)
                vector._custom_dve(
                    op,
                    out=scr_d[:, :w],
                    in0=xbuf[:, xp_off[k] : xp_off[k + 1]],
                    in1=sbuf[:, xp_off[k] : xp_off[k + 1]],
                    s0=1.75,
                    s1=8.0,
                    imm2=8.0,
                    accum_out=acc[:, k : k + 1],
                ).then_inc(dve, 1)

        @block.gpsimd
        def _(gpsimd):
            gpsimd.memset(sig_bias[:, :], -480.0).then_inc(bias_rdy, 1)

        @block.scalar
        def _(scalar):
            # warmup: pull the sigmoid table in while the first DMA lands
            scalar.activation(scr_a[:, 0:1], scr_a[:, 0:1], Act.Sigmoid)
            scalar.wait_ge(bias_rdy, 1)
            for j in range(ns):
                w = s_cols[j]
                scalar.wait_ge(ds[j], 16)
                scalar.activation(
                    scr_a[:, :w],
                    sbuf[:, sp_off[j] : sp_off[j + 1]],
                    Act.Sigmoid,
                    bias=sig_bias[:, :],
                    scale=64.0,
                    accum_out=acc[:, nd + j : nd + j + 1],
                ).then_inc(act, 1)
            scalar.wait_ge(mm, 1)
            scalar.activation(red[0:1, :], ps.ap()[0:1, :], Act.Copy).then_inc(act, 1)
            scalar.wait_ge(act, ns + 1)
            scalar.dma_start(out=out_ext.ap()[:], in_=red[0:1, :]).then_inc(outd, 16)

        @block.tensor
        def _(tensor):
            tensor.wait_ge(dve, nd)
            tensor.wait_ge(act, ns)
            tensor.matmul(
                ps.ap()[0:1, :], ones_f, acc[:, :], start=True, stop=True
            ).then_inc(mm, 1)

    nc.finalize()
    return nc


def build_v5(shard=SHARD, x_cols=None, s_cols=None):
    """v4 + faster ramp and tail.

    - x DMAs on the sync HWDGE ring, s DMAs on the scalar HWDGE ring
      (two rings run concurrently; both streams ramp together)
    - tiny first chunks so DVE/ACT start ~3us earlier
    - no TensorE/PSUM final reduction: the idle sync engine ships the raw
      [128, nd+ns] f32 accumulator tile; host does the final 1.5 KB sum
    - only 3 engine programs (sync/vector/scalar) -> less preamble work
    """
    import concourse.bacc as bacc
    from concourse import mybir

    op = _get_sse_op()

    free = shard // P
    if x_cols is None:
        x_cols = [512, 1024, 2304, 2816, 2816, 2816, 2560, 1536]
    if s_cols is None:
        s_cols = [1024, 3072, 6144, 6144]
    if sum(x_cols) != free:
        x_cols = [free // 8] * 8
    if sum(s_cols) != free:
        s_cols = [free // 4] * 4
    assert sum(x_cols) == free and sum(s_cols) == free
    nd, ns = len(x_cols), len(s_cols)
    xp_off = [sum(x_cols[:i]) for i in range(nd + 1)]
    sp_off = [sum(s_cols[:i]) for i in range(ns + 1)]
    s_for_x = [next(j for j in range(ns) if sp_off[j + 1] >= xp_off[k + 1])
               for k in range(nd)]

    nc = bacc.Bacc("TRN2", target_bir_lowering=False)
    bf16 = mybir.dt.bfloat16
    f32 = mybir.dt.float32
    fp8 = mybir.dt.float8e4
    Act = mybir.ActivationFunctionType

    x_ext = nc.declare_dram_parameter("blast_scores", [shard], bf16, isOutput=False)
    s_ext = nc.declare_dram_parameter("stage_labels", [shard], fp8, isOutput=False)
    out_ext = nc.declare_dram_parameter("out", [P * (nd + ns)], f32, isOutput=True)

    x_v = x_ext.ap().rearrange("(p f) -> p f", p=P)
    s_v = s_ext.ap().rearrange("(p f) -> p f", p=P)
    out_v = out_ext.ap().rearrange("(p f) -> p f", p=P)

    xbuf = nc.alloc_sbuf_tensor("xbuf", [P, free], bf16).ap()
    sbuf = nc.alloc_sbuf_tensor("sbuf", [P, free], fp8).ap()
    scr_d = nc.alloc_sbuf_tensor("scr_d", [P, max(x_cols)], bf16).ap()
    scr_a = nc.alloc_sbuf_tensor("scr_a", [P, max(s_cols)], fp8).ap()
    acc = nc.alloc_sbuf_tensor("acc", [P, nd + ns], f32).ap()
    sig_bias = nc.alloc_sbuf_tensor("sig_bias", [P, 1], f32).ap()

    from contextlib import ExitStack

    with ExitStack() as ctx:
        dx = [ctx.enter_context(nc.semaphore(f"dx{i}")) for i in range(nd)]
        ds = [ctx.enter_context(nc.semaphore(f"ds{j}")) for j in range(ns)]
        dve = ctx.enter_context(nc.semaphore("dve"))
        act = ctx.enter_context(nc.semaphore("act"))
        outd = ctx.enter_context(nc.semaphore("outd"))
        bias_rdy = ctx.enter_context(nc.semaphore("bias_rdy"))
        block = ctx.enter_context(nc.Block())

        @block.sync
        def _(sync):
            for i in range(nd):
                sync.dma_start(
                    out=xbuf[:, xp_off[i] : xp_off[i + 1]],
                    in_=x_v[:, xp_off[i] : xp_off[i + 1]],
                ).then_inc(dx[i], 16)
            sync.wait_ge(dve, nd)
            sync.wait_ge(act, ns)
            sync.dma_start(out=out_v[:, :], in_=acc[:, :]).then_inc(outd, 16)

        @block.vector
        def _(vector):
            vector.memset(sig_bias[:, :], -480.0).then_inc(bias_rdy, 1)
            for k in range(nd):
                w = x_cols[k]
                vector.wait_ge(ds[s_for_x[k]], 16)
                vector.wait_ge(dx[k], 16)
                vector._custom_dve(
                    op,
                    out=scr_d[:, :w],
                    in0=xbuf[:, xp_off[k] : xp_off[k + 1]],
                    in1=sbuf[:, xp_off[k] : xp_off[k + 1]],
                    s0=1.75,
                    s1=8.0,
                    imm2=8.0,
                    accum_out=acc[:, k : k + 1],
                ).then_inc(dve, 1)

        @block.scalar
        def _(scalar):
            for j in range(ns):
                scalar.dma_start(
                    out=sbuf[:, sp_off[j] : sp_off[j + 1]],
                    in_=s_v[:, sp_off[j] : sp_off[j + 1]],
                ).then_inc(ds[j], 16)
            # warmup: pull the sigmoid table in while the first DMA lands
            scalar.activation(scr_a[:, 0:1], scr_a[:, 0:1], Act.Sigmoid)
            scalar.wait_ge(bias_rdy, 1)
            for j in range(ns):
                w = s_cols[j]
                scalar.wait_ge(ds[j], 16)
                scalar.activation(
                    scr_a[:, :w],
                    sbuf[:, sp_off[j] : sp_off[j + 1]],
                    Act.Sigmoid,
                    bias=sig_bias[:, :],
                    scale=64.0,
                    accum_out=acc[:, nd + j : nd + j + 1],
                ).then_inc(act, 1)

    nc.finalize()
    return nc


def build_v6(shard=SHARD, x_cols=None, s_cols=None):
    """v5 + both inputs staged fp8e3 (e3m4): 4 MB/core HBM traffic.

    The custom DVE op runs at 1x regardless of src dtype, so fp8 scores are
    free on the compute side (rel err ~2e-5 vs 7e-7 at bf16 -- gate is 2e-2).
    With DMA (~13 us) far under DVE (~18 us), a single sync-ring stream
    ordered by consumption keeps DVE gapless from the first tile.
    """
    import concourse.bacc as bacc
    from concourse import mybir

    op = _get_sse_op()

    free = shard // P
    if x_cols is None:
        x_cols = [512, 2048, 3072, 3584, 3584, 3584]
    if s_cols is None:
        s_cols = [2048, 4096, 5120, 5120]
    if sum(x_cols) != free:
        x_cols = [free // 8] * 8
    if sum(s_cols) != free:
        s_cols = [free // 4] * 4
    assert sum(x_cols) == free and sum(s_cols) == free
    nd, ns = len(x_cols), len(s_cols)
    xp_off = [sum(x_cols[:i]) for i in range(nd + 1)]
    sp_off = [sum(s_cols[:i]) for i in range(ns + 1)]
    s_for_x = [next(j for j in range(ns) if sp_off[j + 1] >= xp_off[k + 1])
               for k in range(nd)]

    nc = bacc.Bacc("TRN2", target_bir_lowering=False)
    f32 = mybir.dt.float32
    fp8 = mybir.dt.float8e3
    Act = mybir.ActivationFunctionType

    x_ext = nc.declare_dram_parameter("blast_scores", [shard], fp8, isOutput=False)
    s_ext = nc.declare_dram_parameter("stage_labels", [shard], fp8, isOutput=False)
    out_ext = nc.declare_dram_parameter("out", [P * (nd + ns)], f32, isOutput=True)

    x_v = x_ext.ap().rearrange("(p f) -> p f", p=P)
    s_v = s_ext.ap().rearrange("(p f) -> p f", p=P)
    out_v = out_ext.ap().rearrange("(p f) -> p f", p=P)

    xbuf = nc.alloc_sbuf_tensor("xbuf", [P, free], fp8).ap()
    sbuf = nc.alloc_sbuf_tensor("sbuf", [P, free], fp8).ap()
    scr_d = nc.alloc_sbuf_tensor("scr_d", [P, max(x_cols)], mybir.dt.bfloat16).ap()
    scr_a = nc.alloc_sbuf_tensor("scr_a", [P, max(s_cols)], fp8).ap()
    acc = nc.alloc_sbuf_tensor("acc", [P, nd + ns], f32).ap()
    sig_bias = nc.alloc_sbuf_tensor("sig_bias", [P, 1], f32).ap()

    from contextlib import ExitStack

    with ExitStack() as ctx:
        dx = [ctx.enter_context(nc.semaphore(f"dx{i}")) for i in range(nd)]
        ds = [ctx.enter_context(nc.semaphore(f"ds{j}")) for j in range(ns)]
        dve = ctx.enter_context(nc.semaphore("dve"))
        act = ctx.enter_context(nc.semaphore("act"))
        outd = ctx.enter_context(nc.semaphore("outd"))
        bias_rdy = ctx.enter_context(nc.semaphore("bias_rdy"))
        block = ctx.enter_context(nc.Block())

        # single-ring issue order: each s chunk right before the first x
        # chunk that needs it; s0 first so ACT can start asap
        issue = []
        xi = 0
        for j in range(ns):
            issue.append(("s", j))
            while xi < nd and s_for_x[xi] <= j:
                issue.append(("x", xi)); xi += 1

        @block.sync
        def _(sync):
            for kind, i in issue:
                if kind == "s":
                    sync.dma_start(
                        out=sbuf[:, sp_off[i] : sp_off[i + 1]],
                        in_=s_v[:, sp_off[i] : sp_off[i + 1]],
                    ).then_inc(ds[i], 16)
                else:
                    sync.dma_start(
                        out=xbuf[:, xp_off[i] : xp_off[i + 1]],
                        in_=x_v[:, xp_off[i] : xp_off[i + 1]],
                    ).then_inc(dx[i], 16)
            sync.wait_ge(dve, nd)
            sync.wait_ge(act, ns)
            sync.dma_start(out=out_v[:, :], in_=acc[:, :]).then_inc(outd, 16)

        @block.vector
        def _(vector):
            vector.memset(sig_bias[:, :], -480.0).then_inc(bias_rdy, 1)
            for k in range(nd):
                w = x_cols[k]
                vector.wait_ge(ds[s_for_x[k]], 16)
                vector.wait_ge(dx[k], 16)
                vector._custom_dve(
                    op,
                    out=scr_d[:, :w],
                    in0=xbuf[:, xp_off[k] : xp_off[k + 1]],
                    in1=sbuf[:, xp_off[k] : xp_off[k + 1]],
                    s0=1.75,
                    s1=8.0,
                    imm2=8.0,
                    accum_out=acc[:, k : k + 1],
                ).then_inc(dve, 1)

        @block.scalar
        def _(scalar):
            # warmup first: sigmoid tables load while the first DMAs land
            scalar.activation(scr_a[:, 0:1], scr_a[:, 0:1], Act.Sigmoid)
            scalar.wait_ge(bias_rdy, 1)
            for j in range(ns):
                w = s_cols[j]
                scalar.wait_ge(ds[j], 16)
                scalar.activation(
                    scr_a[:, :w],
                    sbuf[:, sp_off[j] : sp_off[j + 1]],
                    Act.Sigmoid,
                    bias=sig_bias[:, :],
                    scale=64.0,
                    accum_out=acc[:, nd + j : nd + j + 1],
                ).then_inc(act, 1)

    nc.finalize()
    return nc


def build_v7(shard=SHARD, x_cols=None, s_cols=None):
    """v6 + three parallel DMA paths so DVE is never starved.

    sync HWDGE ring: x chunks only; gpsimd SWDGE ring: s chunks; scalar:
    pure ACT (tables load at t0).  Both inputs fp8e3 (4 MB/core).
    """
    import concourse.bacc as bacc
    from concourse import mybir

    op = _get_sse_op()

    free = shard // P
    if x_cols is None:
        x_cols = [512, 1536, 2048, 2048, 2560, 2560, 2560, 2560]
    if s_cols is None:
        s_cols = [4096, 6144, 6144]
    if sum(x_cols) != free:
        x_cols = [free // 8] * 8
    if sum(s_cols) != free:
        s_cols = [free // 4] * 4
    assert sum(x_cols) == free and sum(s_cols) == free
    nd, ns = len(x_cols), len(s_cols)
    xp_off = [sum(x_cols[:i]) for i in range(nd + 1)]
    sp_off = [sum(s_cols[:i]) for i in range(ns + 1)]
    s_for_x = [next(j for j in range(ns) if sp_off[j + 1] >= xp_off[k + 1])
               for k in range(nd)]

    nc = bacc.Bacc("TRN2", target_bir_lowering=False)
    f32 = mybir.dt.float32
    fp8 = mybir.dt.float8e3
    Act = mybir.ActivationFunctionType

    x_ext = nc.declare_dram_parameter("blast_scores", [shard], fp8, isOutput=False)
    s_ext = nc.declare_dram_parameter("stage_labels", [shard], fp8, isOutput=False)
    out_ext = nc.declare_dram_parameter("out", [P * (nd + ns)], f32, isOutput=True)

    x_v = x_ext.ap().rearrange("(p f) -> p f", p=P)
    s_v = s_ext.ap().rearrange("(p f) -> p f", p=P)
    out_v = out_ext.ap().rearrange("(p f) -> p f", p=P)

    xbuf = nc.alloc_sbuf_tensor("xbuf", [P, free], fp8).ap()
    sbuf = nc.alloc_sbuf_tensor("sbuf", [P, free], fp8).ap()
    scr_d = nc.alloc_sbuf_tensor("scr_d", [P, max(x_cols)], mybir.dt.bfloat16).ap()
    scr_a = nc.alloc_sbuf_tensor("scr_a", [P, max(s_cols)], fp8).ap()
    acc = nc.alloc_sbuf_tensor("acc", [P, nd + ns], f32).ap()
    sig_bias = nc.alloc_sbuf_tensor("sig_bias", [P, 1], f32).ap()

    from contextlib import ExitStack

    with ExitStack() as ctx:
        dx = [ctx.enter_context(nc.semaphore(f"dx{i}")) for i in range(nd)]
        ds = [ctx.enter_context(nc.semaphore(f"ds{j}")) for j in range(ns)]
        dve = ctx.enter_context(nc.semaphore("dve"))
        act = ctx.enter_context(nc.semaphore("act"))
        outd = ctx.enter_context(nc.semaphore("outd"))
        bias_rdy = ctx.enter_context(nc.semaphore("bias_rdy"))
        block = ctx.enter_context(nc.Block())

        @block.sync
        def _(sync):
            for i in range(nd):
                sync.dma_start(
                    out=xbuf[:, xp_off[i] : xp_off[i + 1]],
                    in_=x_v[:, xp_off[i] : xp_off[i + 1]],
                ).then_inc(dx[i], 16)
            sync.wait_ge(dve, nd)
            sync.wait_ge(act, ns)
            sync.dma_start(out=out_v[:, :], in_=acc[:, :]).then_inc(outd, 16)

        @block.gpsimd
        def _(gpsimd):
            gpsimd.memset(sig_bias[:, :], -480.0).then_inc(bias_rdy, 1)
            for j in range(ns):
                gpsimd.dma_start(
                    out=sbuf[:, sp_off[j] : sp_off[j + 1]],
                    in_=s_v[:, sp_off[j] : sp_off[j + 1]],
                ).then_inc(ds[j], 16)

        @block.vector
        def _(vector):
            for k in range(nd):
                w = x_cols[k]
                vector.wait_ge(ds[s_for_x[k]], 16)
                vector.wait_ge(dx[k], 16)
                vector._custom_dve(
                    op,
                    out=scr_d[:, :w],
                    in0=xbuf[:, xp_off[k] : xp_off[k + 1]],
                    in1=sbuf[:, xp_off[k] : xp_off[k + 1]],
                    s0=1.75,
                    s1=8.0,
                    imm2=8.0,
                    accum_out=acc[:, k : k + 1],
                ).then_inc(dve, 1)

        @block.scalar
        def _(scalar):
            # warmup first: sigmoid tables load while the first DMAs land
            scalar.activation(scr_a[:, 0:1], scr_a[:, 0:1], Act.Sigmoid)
            scalar.wait_ge(bias_rdy, 1)
            for j in range(ns):
                w = s_cols[j]
                scalar.wait_ge(ds[j], 16)
                scalar.activation(
                    scr_a[:, :w],
                    sbuf[:, sp_off[j] : sp_off[j + 1]],
                    Act.Sigmoid,
                    bias=sig_bias[:, :],
                    scale=64.0,
                    accum_out=acc[:, nd + j : nd + j + 1],
                ).then_inc(act, 1)

    nc.finalize()
    return nc


def build_v8(shard=SHARD, cols=None, act_groups=2):
    """Interleaved single-stream design.

    Host interleaves x and s into one fp8e3 array [x0,s0,x1,s1,...]; each
    DMA chunk carries both tensors for its span, so one semaphore gates
    both consumers and the x/s delivery ratio is always right.  Chunks
    alternate between the sync and scalar HWDGE rings (2 in flight).
    DVE reads stride-2 views (1x mode is stride-agnostic); ACT sigmoid
    reads the stride-2 s view.
    """
    import concourse.bacc as bacc
    from concourse import mybir

    op = _get_sse_op()

    free = shard // P  # pairs per partition
    if cols is None:
        cols = [512, 1536, 2048, 2048, 2560, 2560, 2560, 2560]  # pair counts
    if sum(cols) != free:
        cols = [free // 8] * 8
    assert sum(cols) == free
    nd = len(cols)
    off = [sum(cols[:i]) for i in range(nd + 1)]
    # ACT op j covers chunks [j*act_groups, (j+1)*act_groups)
    assert nd % act_groups == 0
    ns = nd // act_groups

    nc = bacc.Bacc("TRN2", target_bir_lowering=False)
    f32 = mybir.dt.float32
    fp8 = mybir.dt.float8e3
    Act = mybir.ActivationFunctionType

    xs_ext = nc.declare_dram_parameter("xs", [2 * shard], fp8, isOutput=False)
    out_ext = nc.declare_dram_parameter("out", [P * (nd + ns)], f32, isOutput=True)

    xs_v = xs_ext.ap().rearrange("(p f two) -> p f two", p=P, two=2)
    out_v = out_ext.ap().rearrange("(p f) -> p f", p=P)

    ibuf = nc.alloc_sbuf_tensor("ibuf", [P, free, 2], fp8).ap()
    x_view = ibuf[:, :, 0]
    s_view = ibuf[:, :, 1]
    scr_d = nc.alloc_sbuf_tensor("scr_d", [P, max(cols)], mybir.dt.bfloat16).ap()
    scr_a = nc.alloc_sbuf_tensor("scr_a", [P, act_groups * max(cols)], fp8).ap()
    acc = nc.alloc_sbuf_tensor("acc", [P, nd + ns], f32).ap()
    sig_bias = nc.alloc_sbuf_tensor("sig_bias", [P, 1], f32).ap()

    from contextlib import ExitStack

    with ExitStack() as ctx:
        dc = [ctx.enter_context(nc.semaphore(f"dc{i}")) for i in range(nd)]
        dve = ctx.enter_context(nc.semaphore("dve"))
        act = ctx.enter_context(nc.semaphore("act"))
        outd = ctx.enter_context(nc.semaphore("outd"))
        bias_rdy = ctx.enter_context(nc.semaphore("bias_rdy"))
        block = ctx.enter_context(nc.Block())

        @block.sync
        def _(sync):
            for i in range(0, nd, 2):
                sync.dma_start(
                    out=ibuf[:, off[i] : off[i + 1], :],
                    in_=xs_v[:, off[i] : off[i + 1], :],
                ).then_inc(dc[i], 16)
            sync.wait_ge(dve, nd)
            sync.wait_ge(act, ns)
            sync.dma_start(out=out_v[:, :], in_=acc[:, :]).then_inc(outd, 16)

        @block.vector
        def _(vector):
            vector.memset(sig_bias[:, :], -480.0).then_inc(bias_rdy, 1)
            for k in range(nd):
                w = cols[k]
                vector.wait_ge(dc[k], 16)
                vector._custom_dve(
                    op,
                    out=scr_d[:, :w],
                    in0=x_view[:, off[k] : off[k + 1]],
                    in1=s_view[:, off[k] : off[k + 1]],
                    s0=1.75,
                    s1=8.0,
                    imm2=8.0,
                    accum_out=acc[:, k : k + 1],
                ).then_inc(dve, 1)

        @block.scalar
        def _(scalar):
            for i in range(1, nd, 2):
                scalar.dma_start(
                    out=ibuf[:, off[i] : off[i + 1], :],
                    in_=xs_v[:, off[i] : off[i + 1], :],
                ).then_inc(dc[i], 16)
            # warmup: sigmoid tables load while the first chunks land
            scalar.activation(scr_a[:, 0:1], scr_a[:, 0:1], Act.Sigmoid)
            scalar.wait_ge(bias_rdy, 1)
            for j in range(ns):
                lo, hi = off[j * act_groups], off[(j + 1) * act_groups]
                for g in range(j * act_groups, (j + 1) * act_groups):
                    scalar.wait_ge(dc[g], 16)
                scalar.activation(
                    scr_a[:, : hi - lo],
                    s_view[:, lo:hi],
                    Act.Sigmoid,
                    bias=sig_bias[:, :],
                    scale=64.0,
                    accum_out=acc[:, nd + j : nd + j + 1],
                ).then_inc(act, 1)

    nc.finalize()
    return nc


def _to_bf16(a):
    import ml_dtypes

    return np.ascontiguousarray(a.astype(ml_dtypes.bfloat16))


def _to_fp8(a):
    import ml_dtypes

    return np.ascontiguousarray(a.astype(np.float32).astype(ml_dtypes.float8_e4m3fn))


def _to_fp8e3(a):
    import ml_dtypes

    return np.ascontiguousarray(a.astype(np.float32).astype(ml_dtypes.float8_e3m4))


def run(x, s, variant="v3", **spmd_kwargs):
    """Shard, run on 8 cores, host-reduce. Returns (loss, BassKernelResults)."""
    from concourse.bass_utils import run_bass_kernel_spmd

    if variant not in _NC_CACHE:
        if variant == "raw":
            _NC_CACHE[variant] = build_raw()
        elif variant == "v3":
            _NC_CACHE[variant] = build_v3()
        elif variant == "v4":
            _NC_CACHE[variant] = build_v4()
        elif variant == "v5":
            _NC_CACHE[variant] = build_v5()
        elif variant == "v6":
            _NC_CACHE[variant] = build_v6()
        elif variant == "v7":
            _NC_CACHE[variant] = build_v7()
        elif variant == "v8":
            _NC_CACHE[variant] = build_v8()
        else:
            raise ValueError(variant)
    nc = _NC_CACHE[variant]

    if variant == "v8":
        import ml_dtypes

        inter = np.empty(2 * B, dtype=ml_dtypes.float8_e3m4)
        inter[0::2] = x.astype(ml_dtypes.float8_e3m4)
        inter[1::2] = s.astype(np.float32).astype(ml_dtypes.float8_e3m4)
        in_maps = [
            {"xs": inter[i * 2 * SHARD : (i + 1) * 2 * SHARD]}
            for i in range(N_CORES)
        ]
        res = run_bass_kernel_spmd(
            nc, in_maps, core_ids=list(range(N_CORES)), **spmd_kwargs
        )
        cnt = 0.0
        sse = 0.0
        for r in res.results:
            o = r["out"].astype(np.float64).reshape(P, 12)
            sse += o[:, :8].sum() / 3.0625
            cnt += o[:, 8:].sum()
        val = sse / max(cnt, 1.0) if cnt > 0 else 0.0
        return np.asarray(val, dtype=np.float32), res

    if variant == "raw":
        xs, ss = x, s
    elif variant in ("v6", "v7"):
        xs, ss = _to_fp8e3(x), _to_fp8e3(s)
    elif variant in ("v4", "v5"):
        xs, ss = _to_bf16(x), _to_fp8(s)
    else:
        xs, ss = _to_bf16(x), _to_bf16(s)

    in_maps = [
        {
            "blast_scores": xs[i * SHARD : (i + 1) * SHARD],
            "stage_labels": ss[i * SHARD : (i + 1) * SHARD],
        }
        for i in range(N_CORES)
    ]
    res = run_bass_kernel_spmd(nc, in_maps, core_ids=list(range(N_CORES)), **spmd_kwargs)

    cnt = 0.0
    sse = 0.0
    for r in res.results:
        o = r["out"].astype(np.float64)
        if variant == "raw":
            o = o.reshape(2, -1)
            cnt += o[0].sum()
            sse += o[1].sum()
        elif variant == "v4":
            sse += o[:8].sum() / 3.0625  # undo the 1.75^2 prescale
            cnt += o[8:].sum()
        elif variant == "v5":
            o = o.reshape(P, 12)
            sse += o[:, :8].sum() / 3.0625
            cnt += o[:, 8:].sum()
        elif variant == "v6":
            o = o.reshape(P, 10)
            sse += o[:, :6].sum() / 3.0625
            cnt += o[:, 6:].sum()
        elif variant == "v7":
            o = o.reshape(P, 11)
            sse += o[:, :8].sum() / 3.0625
            cnt += o[:, 8:].sum()
        else:
            cnt += o[:512].sum()
            sse += o[512:].sum()
    val = sse / max(cnt, 1.0) if cnt > 0 else 0.0
    return np.asarray(val, dtype=np.float32), res


def kernel(**inputs):
    x = np.ascontiguousarray(np.asarray(inputs["blast_scores"], dtype=np.float32))
    s = np.ascontiguousarray(np.asarray(inputs["stage_labels"], dtype=np.int32))
    assert x.shape == (B,) and s.shape == (B,)
    return run(x, s)[0]



# revision 23
# speedup vs baseline: 1.1056x; 1.0850x over previous
"""Bass/Trainium2 kernel for nn_BlastocystAuxLoss.

Computes a masked MSE over B=16,777,216 elements:
    late stages are labels 8..15; target[s] = (s-8) * 4/7 for late stages;
    loss = sum_{s>=8} (x - target)^2 / count(s>=8)   (0.0 if count == 0)

Strategy: trivially data-parallel over 8 NeuronCores. Inputs are staged to
the device as bf16 (labels 0..15 are exact in bf16; scores were already
rounded to bf16 inside the original f32 kernel's DVE ops, so accuracy is
unchanged at ~3e-6 rel err) which halves HBM traffic to 8 MB per core.
Each core computes per-partition partial {count, sse} on-chip and ships a
tiny [520] f32 partials row; the final reduction (sum + divide) happens on
host in f64. No collectives needed.

Engine split (see build_v3 for the measured perf-mode rules it encodes):
    DVE: xp = 1.75*x+8 [4x], m = (s>=8) [4x], w0 = xp-s [2x], wm = w0*m [2x]
    ACT: sq = Square(4/7*wm) with free accum -> sse   (exact 0 when m=0)
    TE : ones^T @ m in 512-col chunks -> PSUM -> count
"""

from contextlib import ExitStack

import numpy as np

B = 16777216
N_CORES = 8
SHARD = B // N_CORES  # 2,097,152
P = 128

_NC_CACHE = {}


def build(shard=SHARD, n_tiles=8):
    """Build the single-core Bass program (same SPMD program for all cores)."""
    import concourse.bacc as bacc
    import concourse.tile as tile
    from concourse import mybir

    free = shard // P
    fd = free // n_tiles
    assert fd * n_tiles * P == shard

    nc = bacc.Bacc("TRN2", target_bir_lowering=False)
    x_ext = nc.declare_dram_parameter(
        "blast_scores", [shard], mybir.dt.float32, isOutput=False
    )
    s_ext = nc.declare_dram_parameter(
        "stage_labels", [shard], mybir.dt.int32, isOutput=False
    )
    out_ext = nc.declare_dram_parameter("out", [P, 2], mybir.dt.float32, isOutput=True)

    x_v = x_ext.ap().rearrange("(p f) -> p f", p=P)
    s_v = s_ext.ap().rearrange("(p f) -> p f", p=P)

    c47 = 4.0 / 7.0  # target step; folded into the Square's input scale
    c74 = 7.0 / 4.0  # x prescale so z = 7/4*(x - t) uses integer-exact v

    f32 = mybir.dt.float32
    bf16 = mybir.dt.bfloat16
    Alu = mybir.AluOpType
    Act = mybir.ActivationFunctionType

    with tile.TileContext(nc) as tc:
        with (
            tc.tile_pool(name="io", bufs=4) as io_pool,
            tc.tile_pool(name="mid", bufs=3) as mid_pool,
            tc.tile_pool(name="acc", bufs=1) as acc_pool,
        ):
            cnt_acc = acc_pool.tile([P, n_tiles], f32)
            sse_acc = acc_pool.tile([P, n_tiles], f32)
            red = acc_pool.tile([P, 2], f32)
            # bias for the sigmoid step mask: m = sigmoid(64*s - 480)
            sig_bias = acc_pool.tile([P, 1], f32)
            nc.gpsimd.memset(sig_bias[:], -480.0)

            for k in range(n_tiles):
                x_t = io_pool.tile([P, fd], f32, tag="x")
                s_t = io_pool.tile([P, fd], mybir.dt.int32, tag="s")
                nc.sync.dma_start(out=x_t[:], in_=x_v[:, k * fd : (k + 1) * fd])
                nc.sync.dma_start(out=s_t[:], in_=s_v[:, k * fd : (k + 1) * fd])

                m = mid_pool.tile([P, fd], bf16, tag="m")
                v = mid_pool.tile([P, fd], bf16, tag="v")
                z = mid_pool.tile([P, fd], bf16, tag="z")
                zm = mid_pool.tile([P, fd], bf16, tag="zm")
                sq = mid_pool.tile([P, fd], bf16, tag="sq")

                # ACT: step mask m = sigmoid(64*(s - 7.5)) in {0,1} exactly
                # (saturated at +-32); accumulate count for free
                nc.scalar.activation(
                    m[:], s_t[:], Act.Sigmoid, bias=sig_bias[:], scale=64.0,
                    accum_out=cnt_acc[:, k : k + 1],
                )
                # DVE: v = max(s-8, 0)
                nc.vector.tensor_scalar(v[:], s_t[:], 8, 0, Alu.subtract, Alu.max)
                # DVE: z = 7/4*x - v  (== 7/4*(x - target) since v = 7/4*t)
                nc.vector.scalar_tensor_tensor(
                    z[:], x_t[:], c74, v[:], Alu.mult, Alu.subtract
                )
                nc.vector.tensor_tensor(zm[:], z[:], m[:], Alu.mult)
                # ACT: sse += (4/7 * zm)^2 over masked elements
                nc.scalar.activation(
                    sq[:], zm[:], Act.Square, scale=c47,
                    accum_out=sse_acc[:, k : k + 1],
                )

            nc.vector.reduce_sum(red[:, 0:1], cnt_acc[:], axis=mybir.AxisListType.X)
            nc.vector.reduce_sum(red[:, 1:2], sse_acc[:], axis=mybir.AxisListType.X)
            nc.sync.dma_start(out=out_ext.ap()[:, :], in_=red[:])

    nc.finalize()
    return nc


def build_raw(shard=2097152, sizes=None, ring=6):
    """Hand-scheduled raw-Bass builder (no TileContext).

    - per-slot DMA semaphores (multi-queue completions are unordered);
      slot reuse (tile k vs k+R) is ordered by issue-side consumer waits
    - ring of 6 slots so DMA issue never gates on compute and the input
      stream stays bandwidth-bound end to end
    - tile sizes taper at the end so the last tile's compute lag after
      the final (bandwidth-bound) DMA is minimal
    - final reduction via a TensorEngine ones-matmul (cross-partition sum
      -> PSUM [1, 2*NT]) so the output DMA is one small descriptor instead
      of 128 8-byte ones
    """
    import concourse.bacc as bacc
    from concourse import mybir

    free = shard // P
    if sizes is None:
        sizes = [2048] * 7 + [1536, 512]
        if sum(sizes) != free:  # non-default shard (tests)
            fd = free // 8
            sizes = [fd] * 8
    assert sum(sizes) == free
    fd = max(sizes)
    NT = len(sizes)
    offs = [sum(sizes[:i]) for i in range(NT)]
    R = min(ring, NT)

    nc = bacc.Bacc("TRN2", target_bir_lowering=False)
    x_ext = nc.declare_dram_parameter(
        "blast_scores", [shard], mybir.dt.float32, isOutput=False
    )
    s_ext = nc.declare_dram_parameter(
        "stage_labels", [shard], mybir.dt.int32, isOutput=False
    )
    out_ext = nc.declare_dram_parameter("out", [2 * NT], mybir.dt.float32, isOutput=True)

    x_v = x_ext.ap().rearrange("(p f) -> p f", p=P)
    s_v = s_ext.ap().rearrange("(p f) -> p f", p=P)

    c47 = 4.0 / 7.0
    c74 = 7.0 / 4.0

    f32 = mybir.dt.float32
    i32 = mybir.dt.int32
    bf16 = mybir.dt.bfloat16
    Alu = mybir.AluOpType
    Act = mybir.ActivationFunctionType

    x_t = [nc.alloc_sbuf_tensor(f"x{i}", [P, fd], f32).ap() for i in range(R)]
    s_t = [nc.alloc_sbuf_tensor(f"s{i}", [P, fd], i32).ap() for i in range(R)]
    m_t = [nc.alloc_sbuf_tensor(f"m{i}", [P, fd], bf16).ap() for i in range(R)]
    v_t = [nc.alloc_sbuf_tensor(f"v{i}", [P, fd], bf16).ap() for i in range(2)]
    z_t = [nc.alloc_sbuf_tensor(f"z{i}", [P, fd], bf16).ap() for i in range(2)]
    zm_t = [nc.alloc_sbuf_tensor(f"zm{i}", [P, fd], bf16).ap() for i in range(R)]
    sq_t = nc.alloc_sbuf_tensor("sq", [P, fd], bf16).ap()
    # acc[:, k] = per-partition count of tile k; acc[:, NT+k] = partial sse
    acc = nc.alloc_sbuf_tensor("acc", [P, 2 * NT], f32).ap()
    red1 = nc.alloc_sbuf_tensor("red1", [1, 2 * NT], f32).ap()
    sig_bias = nc.alloc_sbuf_tensor("sig_bias", [P, 1], f32).ap()
    ones = nc.const_aps.tensor(1.0, (P, 1), f32)

    with ExitStack() as ctx:
        dma_x = [ctx.enter_context(nc.semaphore(f"dma_x{i}")) for i in range(R)]
        dma_s = [ctx.enter_context(nc.semaphore(f"dma_s{i}")) for i in range(R)]
        dve = ctx.enter_context(nc.semaphore("dve"))
        act = ctx.enter_context(nc.semaphore("act"))
        mm = ctx.enter_context(nc.semaphore("mm"))
        outd = ctx.enter_context(nc.semaphore("outd"))
        bias_rdy = ctx.enter_context(nc.semaphore("bias_rdy"))
        psum = ctx.enter_context(nc.psum_tensor("ps", [1, 2 * NT], f32))
        block = ctx.enter_context(nc.Block())

        # Semaphore increment ledger:
        #   DVE: 3 per tile (v, z, zm)            -> 3*NT total
        #   ACT: 2 per tile (m, sq) + final copy  -> 2*NT + 1 total
        #   DMA slot sems: +16 per transfer into that slot

        @block.sync
        def _(sync):
            for k in range(NT):
                i = k % R
                w = sizes[k]
                if k >= R:
                    # x slot free when z(k-R) done; s slot free when
                    # v(k-R) (implied by z) and m(k-R) done
                    sync.wait_ge(dve, 3 * (k - R) + 2)
                    sync.wait_ge(act, 2 * (k - R) + 1)
                sync.dma_start(
                    out=s_t[i][:, :w], in_=s_v[:, offs[k] : offs[k] + w]
                ).then_inc(dma_s[i], 16)
                sync.dma_start(
                    out=x_t[i][:, :w], in_=x_v[:, offs[k] : offs[k] + w]
                ).then_inc(dma_x[i], 16)
            sync.wait_ge(act, 2 * NT + 1)  # final ScE copy done
            sync.dma_start(out=out_ext.ap()[:], in_=red1[0:1, :]).then_inc(outd, 16)
            if not skip_out_wait:
                sync.wait_ge(outd, 16)

        @block.vector
        def _(vector):
            vector.memset(sig_bias[:, :], -480.0).then_inc(bias_rdy, 1)
            for k in range(NT):
                i = k % R
                w = sizes[k]
                rnd = 16 * (k // R + 1)
                # v = max(s-8, 0)
                vector.wait_ge(dma_s[i], rnd)
                vector.tensor_scalar(
                    v_t[k % 2][:, :w], s_t[i][:, :w], 8, 0, Alu.subtract, Alu.max
                ).then_inc(dve, 1)
                # z = 7/4*x - v
                vector.wait_ge(dma_x[i], rnd)
                vector.wait_ge(dve, 3 * k + 1)  # v(k) drained
                vector.scalar_tensor_tensor(
                    z_t[k % 2][:, :w], x_t[i][:, :w], c74, v_t[k % 2][:, :w],
                    Alu.mult, Alu.subtract,
                ).then_inc(dve, 1)
                # zm = z * m   (m(k) ready when act >= 2k+1)
                vector.wait_ge(act, 2 * k + 1)
                vector.wait_ge(dve, 3 * k + 2)  # z(k) drained
                vector.tensor_tensor(
                    zm_t[i][:, :w], z_t[k % 2][:, :w], m_t[i][:, :w], Alu.mult
                ).then_inc(dve, 1)

        @block.scalar
        def _(scalar):
            scalar.wait_ge(bias_rdy, 1)
            for k in range(NT):
                i = k % R
                w = sizes[k]
                rnd = 16 * (k // R + 1)
                # m = sigmoid(64*s - 480) in {0,1}; count accumulates free
                scalar.wait_ge(dma_s[i], rnd)
                if k >= R:
                    # m slot free when zm(k-R) done
                    scalar.wait_ge(dve, 3 * (k - R) + 3)
                scalar.activation(
                    m_t[i][:, :w], s_t[i][:, :w], Act.Sigmoid,
                    bias=sig_bias[:, :], scale=64.0,
                    accum_out=acc[:, k : k + 1],
                ).then_inc(act, 1)
                # sq = Square(zm * 4/7); sse accum; zm(k): dve >= 3k+3
                scalar.wait_ge(dve, 3 * k + 3)
                scalar.activation(
                    sq_t[:, :w], zm_t[i][:, :w], Act.Square, scale=c47,
                    accum_out=acc[:, NT + k : NT + k + 1],
                ).then_inc(act, 1)
            # after the matmul: PSUM -> SBUF single-partition copy, then
            # ship the 2*NT partials out (single 8*2*NT-byte descriptor);
            # issuing here avoids a cross-engine hop before the final DMA
            scalar.wait_ge(mm, 1)
            scalar.activation(red1[0:1, :], psum.ap()[0:1, :], Act.Copy).then_inc(
                act, 1
            )

        @block.tensor
        def _(tensor):
            # cross-partition reduction: ones.T @ acc -> [1, 2*NT]
            tensor.wait_ge(act, 2 * NT)
            tensor.wait_ge(dve, 3 * NT)
            tensor.matmul(psum.ap()[0:1, :], ones, acc[:, :]).then_inc(mm, 1)

    nc.finalize()
    return nc


def build_v3(shard=SHARD, sizes=None, ring=4):
    """bf16-staged pipeline, fast-mode ops only, x-first tile order.

    Mode rules this is built around (all hardware-measured):
      - DVE tensor_scalar (incl. is_ge): 4x mode; tensor_tensor: 2x;
        scalar_tensor_tensor / accum_out on DVE: 1x (avoided)
      - ACT: 1 elem/cycle/lane, accum_out free -> owns Square + sse
      - TensorE: ones-matmul count accumulation into PSUM (pays ~1.5us of
        DVE SBUF-port contention; cheaper than any accumulating DVE op)
      - output DMAs issued from the Scalar engine (HWDGE); no completion
        wait needed -- the runtime drains DMA queues at NEFF end

    Per element (s = label, x = score, both staged bf16 from host):
      DVE: xp = 1.75*x + 8        [ts 4x]   (x arrives first, so xp leads)
      DVE: m  = (s >= 8)          [ts 4x]
      DVE: w0 = xp - s            [tt 2x]   (masked: 7/4*(x - target))
      DVE: wm = w0 * m            [tt 2x]   (exactly 0 when unmasked)
      ACT: sq = Square(4/7 * wm)  accum -> sse partials
      TE : ones^T @ m chunks -> PSUM[1,512] -> count
    """
    import concourse.bacc as bacc
    from concourse import mybir

    free = shard // P
    if sizes is None:
        sizes = [1024, 1536, 2048, 2560, 3072, 3072, 2560, 512]
        if sum(sizes) != free:  # non-default shard (tests)
            fd = free // 8
            sizes = [fd] * 8
    assert sum(sizes) == free
    fd = max(sizes)
    NT = len(sizes)
    offs = [sum(sizes[:i]) for i in range(NT)]
    R = min(ring, NT)
    CW = 512
    chunks = [
        [(c, min(CW, sizes[k] - c)) for c in range(0, sizes[k], CW)]
        for k in range(NT)
    ]
    cum_ch = [0]
    for k in range(NT):
        cum_ch.append(cum_ch[-1] + len(chunks[k]))
    n_mm = cum_ch[-1] + 1  # + final sse reduction

    nc = bacc.Bacc("TRN2", target_bir_lowering=False)
    bf16 = mybir.dt.bfloat16
    f32 = mybir.dt.float32
    Alu = mybir.AluOpType
    Act = mybir.ActivationFunctionType

    x_ext = nc.declare_dram_parameter("blast_scores", [shard], bf16, isOutput=False)
    s_ext = nc.declare_dram_parameter("stage_labels", [shard], bf16, isOutput=False)
    out_ext = nc.declare_dram_parameter("out", [CW + NT], f32, isOutput=True)

    x_v = x_ext.ap().rearrange("(p f) -> p f", p=P)
    s_v = s_ext.ap().rearrange("(p f) -> p f", p=P)

    x_t = [nc.alloc_sbuf_tensor(f"x{i}", [P, fd], bf16).ap() for i in range(R)]
    s_t = [nc.alloc_sbuf_tensor(f"s{i}", [P, fd], bf16).ap() for i in range(R)]
    xp_t = [nc.alloc_sbuf_tensor(f"xp{i}", [P, fd], bf16).ap() for i in range(2)]
    RM = 3
    m_t = [nc.alloc_sbuf_tensor(f"m{i}", [P, fd], bf16).ap() for i in range(RM)]
    w0_t = [nc.alloc_sbuf_tensor(f"w0{i}", [P, fd], bf16).ap() for i in range(2)]
    RW = 3
    wm_t = [nc.alloc_sbuf_tensor(f"wm{i}", [P, fd], bf16).ap() for i in range(RW)]
    sq_t = nc.alloc_sbuf_tensor("sq", [P, fd], bf16).ap()
    sse_acc = nc.alloc_sbuf_tensor("sse_acc", [P, NT], f32).ap()
    red1 = nc.alloc_sbuf_tensor("red1", [1, CW + NT], f32).ap()
    ones_b = nc.const_aps.tensor(1.0, (P, 1), bf16)
    ones_f = nc.const_aps.tensor(1.0, (P, 1), f32)

    # DVE op retirement offsets within tile k (4 ops/tile):
    XPD, MD, W0D, WMD = 1, 2, 3, 4

    with ExitStack() as ctx:
        dma_x = [ctx.enter_context(nc.semaphore(f"dma_x{i}")) for i in range(R)]
        dma_s = [ctx.enter_context(nc.semaphore(f"dma_s{i}")) for i in range(R)]
        dve = ctx.enter_context(nc.semaphore("dve"))
        act = ctx.enter_context(nc.semaphore("act"))
        mm = ctx.enter_context(nc.semaphore("mm"))
        outd = ctx.enter_context(nc.semaphore("outd"))
        ps_cnt = ctx.enter_context(nc.psum_tensor("pscnt", [1, CW], f32))
        ps_sse = ctx.enter_context(nc.psum_tensor("pssse", [1, NT], f32))
        block = ctx.enter_context(nc.Block())

        @block.sync
        def _(sync):
            for k in range(NT):
                i = k % R
                w = sizes[k]
                if k >= R:
                    # x slot freed by xp(k-R); s slot by w0(k-R)
                    sync.wait_ge(dve, 4 * (k - R) + W0D)
                sync.dma_start(
                    out=x_t[i][:, :w], in_=x_v[:, offs[k] : offs[k] + w]
                ).then_inc(dma_x[i], 16)
                sync.dma_start(
                    out=s_t[i][:, :w], in_=s_v[:, offs[k] : offs[k] + w]
                ).then_inc(dma_s[i], 16)

        @block.vector
        def _(vector):
            for k in range(NT):
                i = k % R
                w = sizes[k]
                rnd = 16 * (k // R + 1)
                jm = k % RM
                jw = k % RW
                # xp = 1.75*x + 8  [4x]
                vector.wait_ge(dma_x[i], rnd)
                vector.tensor_scalar(
                    xp_t[k % 2][:, :w], x_t[i][:, :w], 1.75, 8.0, Alu.mult, Alu.add
                ).then_inc(dve, 1)
                # m = (s >= 8)  [4x]
                vector.wait_ge(dma_s[i], rnd)
                if k >= RM:
                    # m slot reused: freed when TE finishes tile k-RM
                    vector.wait_ge(mm, cum_ch[k - RM + 1])
                vector.tensor_scalar(
                    m_t[jm][:, :w], s_t[i][:, :w], 8.0, 0.0, Alu.is_ge, Alu.add
                ).then_inc(dve, 1)
                # w0 = xp - s  [2x]
                vector.tensor_tensor(
                    w0_t[k % 2][:, :w], xp_t[k % 2][:, :w], s_t[i][:, :w],
                    Alu.subtract,
                ).then_inc(dve, 1)
                # wm = w0 * m  [2x]
                if k >= RW:
                    # wm slot reused: freed by ACT sq(k-RW)
                    vector.wait_ge(act, k - RW + 1)
                vector.tensor_tensor(
                    wm_t[jw][:, :w], w0_t[k % 2][:, :w], m_t[jm][:, :w], Alu.mult
                ).then_inc(dve, 1)

        @block.scalar
        def _(scalar):
            c47 = 4.0 / 7.0
            acts = 0
            for k in range(NT):
                w = sizes[k]
                jw = k % RW
                if k == NT - 1:
                    # count matmuls all retire with m(NT-1); ship the count
                    # half of the output while the last tile still computes
                    scalar.wait_ge(mm, n_mm - 1)
                    scalar.activation(
                        red1[0:1, 0:CW], ps_cnt.ap()[0:1, :], Act.Copy
                    ).then_inc(act, 1)
                    acts += 1
                    # the sequencer runs ahead of the ACT datapath: wait for
                    # the copy to land before the DMA reads red1
                    scalar.wait_ge(act, acts)
                    scalar.dma_start(
                        out=out_ext.ap()[0:CW], in_=red1[0:1, 0:CW]
                    ).then_inc(outd, 16)
                scalar.wait_ge(dve, 4 * k + WMD)
                scalar.activation(
                    sq_t[:, :w], wm_t[jw][:, :w], Act.Square, scale=c47,
                    accum_out=sse_acc[:, k : k + 1],
                ).then_inc(act, 1)
                acts += 1
            scalar.wait_ge(mm, n_mm)
            scalar.activation(
                red1[0:1, CW : CW + NT], ps_sse.ap()[0:1, :], Act.Copy
            ).then_inc(act, 1)
            scalar.wait_ge(act, NT + 2)
            # runtime drains DMA queues at NEFF end; no completion wait
            scalar.dma_start(
                out=out_ext.ap()[CW : CW + NT], in_=red1[0:1, CW : CW + NT]
            ).then_inc(outd, 16)

        @block.tensor
        def _(tensor):
            n_done = 0
            for k in range(NT):
                jm = k % RM
                tensor.wait_ge(dve, 4 * k + MD)
                for (c, cw) in chunks[k]:
                    tensor.matmul(
                        ps_cnt.ap()[0:1, 0:cw], ones_b, m_t[jm][:, c : c + cw],
                        start=(n_done == 0), stop=(n_done == cum_ch[-1] - 1),
                    ).then_inc(mm, 1)
                    n_done += 1
            # all sq done: NT sq ops + the hoisted count copy
            tensor.wait_ge(act, NT + 1)
            tensor.matmul(
                ps_sse.ap()[0:1, 0:NT], ones_f, sse_acc[:, :], start=True, stop=True
            ).then_inc(mm, 1)

    nc.finalize()
    return nc


_SSE_OP = None


def _get_sse_op():
    """Register (once) the fused masked-SSE custom DVE op.

    body = ((x*C0 + C1 - s) * (s >= C2))^2, accum_out = per-partition sum.
    With C0=1.75, C1=8, C2=8:  (1.75*(x - t))^2 for late stages, exactly 0
    otherwise (t = (s-8)*4/7, so 1.75*t = s-8).  One 1x DVE instruction per
    tile replaces the xp/m/w0/wm 4-op chain AND the ACT Square pass.
    """
    global _SSE_OP
    if _SSE_OP is not None:
        return _SSE_OP
    from operator import add

    from concourse import dve_ops as _do
    from concourse.dve_spec import C0, C1, C2, Spec, Src0, Src1, lower, sq
    from concourse.dve_uop import DveOpSpec

    def _ref(in0, in1, s0, s1, imm2):
        x = in0.astype(np.float32)
        s = in1.astype(np.float32)
        b = ((x * s0 + s1 - s) * (s >= imm2).astype(np.float32)) ** 2
        b = b.astype(np.float32)
        return b, b.reshape(b.shape[0], -1).sum(axis=-1, keepdims=True)

    spec = Spec(
        body=sq((Src0 * C0 + C1 - Src1) * (Src1 >= C2)),
        accum=add,
        reference=_ref,
    )
    name = "SSE_MASK_ANT"
    shas = {}
    for ver in ("v3", "v4"):
        s = DveOpSpec(name=name, opcode=0, uops=lower(spec, ver=ver), rd1_en=True)
        shas[ver] = s.sha(ver)
    op = _do.DveOp(name, spec, subdim=False, uops_sha=shas)
    if name not in _do._SUB_OPCODE_FOR_NAME:
        _do.OPS.append(op)
        _do.CUSTOM_DVE_SPECS[name] = spec
        _do._SUB_OPCODE_FOR_NAME[name] = max(_do._SUB_OPCODE_FOR_NAME.values()) + 1
    _SSE_OP = op
    return op


def build_v4(shard=SHARD, x_cols=None, s_cols=None):
    """Fused-DVE design: one custom SSE op per x-tile + ACT sigmoid count.

    Per element (x staged bf16, s staged fp8e4 -- both exact enough):
      DVE : sse_acc[:,k] += ((1.75*x + 8 - s) * (s>=8))^2     [1 op/tile, 1x]
      ACT : cnt_acc[:,j] += sigmoid(64*s - 480)  (exact {0,1}) [1 op/s-chunk]
      TE  : ones^T @ acc -> psum[1, nd+ns] (single tiny matmul at the end)
    HBM traffic: 4 MB (x) + 2 MB (s) = 6 MB/core vs 8 MB for the v3 kernel.
    """
    import concourse.bacc as bacc
    from concourse import mybir

    op = _get_sse_op()

    free = shard // P
    if x_cols is None:
        x_cols = [1024, 1536, 2560, 2560, 2560, 2560, 2048, 1536]
    if s_cols is None:
        s_cols = [2048, 5120, 5120, 4096]
    if sum(x_cols) != free:
        nd = 8
        x_cols = [free // nd] * nd
    if sum(s_cols) != free:
        ns = 4
        s_cols = [free // ns] * ns
    assert sum(x_cols) == free and sum(s_cols) == free
    nd, ns = len(x_cols), len(s_cols)
    xp_off = [sum(x_cols[:i]) for i in range(nd + 1)]
    sp_off = [sum(s_cols[:i]) for i in range(ns + 1)]
    # dve tile k needs the s-chunk covering cols [xp_off[k], xp_off[k+1])
    s_for_x = [next(j for j in range(ns) if sp_off[j + 1] >= xp_off[k + 1])
               for k in range(nd)]

    nc = bacc.Bacc("TRN2", target_bir_lowering=False)
    bf16 = mybir.dt.bfloat16
    f32 = mybir.dt.float32
    fp8 = mybir.dt.float8e4
    Act = mybir.ActivationFunctionType

    x_ext = nc.declare_dram_parameter("blast_scores", [shard], bf16, isOutput=False)
    s_ext = nc.declare_dram_parameter("stage_labels", [shard], fp8, isOutput=False)
    out_ext = nc.declare_dram_parameter("out", [nd + ns], f32, isOutput=True)

    x_v = x_ext.ap().rearrange("(p f) -> p f", p=P)
    s_v = s_ext.ap().rearrange("(p f) -> p f", p=P)

    xbuf = nc.alloc_sbuf_tensor("xbuf", [P, free], bf16).ap()
    sbuf = nc.alloc_sbuf_tensor("sbuf", [P, free], fp8).ap()
    scr_d = nc.alloc_sbuf_tensor("scr_d", [P, max(x_cols)], bf16).ap()
    scr_a = nc.alloc_sbuf_tensor("scr_a", [P, max(s_cols)], fp8).ap()
    # acc[:, 0:nd] = per-tile sse partials; acc[:, nd:nd+ns] = count partials
    acc = nc.alloc_sbuf_tensor("acc", [P, nd + ns], f32).ap()
    red = nc.alloc_sbuf_tensor("red", [1, nd + ns], f32).ap()
    sig_bias = nc.alloc_sbuf_tensor("sig_bias", [P, 1], f32).ap()
    ones_f = nc.const_aps.tensor(1.0, (P, 1), f32)

    from contextlib import ExitStack

    with ExitStack() as ctx:
        dx = [ctx.enter_context(nc.semaphore(f"dx{i}")) for i in range(nd)]
        ds = [ctx.enter_context(nc.semaphore(f"ds{j}")) for j in range(ns)]
        dve = ctx.enter_context(nc.semaphore("dve"))
        act = ctx.enter_context(nc.semaphore("act"))
        mm = ctx.enter_context(nc.semaphore("mm"))
        outd = ctx.enter_context(nc.semaphore("outd"))
        bias_rdy = ctx.enter_context(nc.semaphore("bias_rdy"))
        ps = ctx.enter_context(nc.psum_tensor("ps", [1, nd + ns], f32))
        block = ctx.enter_context(nc.Block())

        # interleave x/s chunk issue by byte progress so both streams arrive
        # proportionally (s chunk j before the x tiles that depend on it)
        issue = []
        xi = si = 0
        while xi < nd or si < ns:
            if si < ns and (xi >= nd or si <= s_for_x[min(xi, nd - 1)]):
                issue.append(("s", si)); si += 1
            else:
                issue.append(("x", xi)); xi += 1

        @block.sync
        def _(sync):
            for kind, i in issue:
                if kind == "s":
                    sync.dma_start(
                        out=sbuf[:, sp_off[i] : sp_off[i + 1]],
                        in_=s_v[:, sp_off[i] : sp_off[i + 1]],
                    ).then_inc(ds[i], 16)
                else:
                    sync.dma_start(
                        out=xbuf[:, xp_off[i] : xp_off[i + 1]],
                        in_=x_v[:, xp_off[i] : xp_off[i + 1]],
                    ).then_inc(dx[i], 16)

        @block.vector
        def _(vector):
            for k in range(nd):
                w = x_cols[k]
                vector.wait_ge(ds[s_for_x[k]], 16)
                vector.wait_ge(dx[k], 16)
                vector._custom_dve(
                    op,
                    out=scr_d[:, :w],
                    in0=xbuf[:, xp_off[k] : xp_off[k + 1]],
                    in1=sbuf[:, xp_off[k] : xp_off[k + 1]],
                    s0=1.75,
                    s1=8.0,
                    imm2=8.0,
                    accum_out=acc[:, k : k + 1],
                ).then_inc(dve, 1)

        @block.gpsimd
        def _(gpsimd):
            gpsimd.memset(sig_bias[:, :], -480.0).then_inc(bias_rdy, 1)

        @block.scalar
        def _(scalar):
            # warmup: pull the sigmoid table in while the first DMA lands
            scalar.activation(scr_a[:, 0:1], scr_a[:, 0:1], Act.Sigmoid)
            scalar.wait_ge(bias_rdy, 1)
            for j in range(ns):
                w = s_cols[j]
                scalar.wait_ge(ds[j], 16)
                scalar.activation(
                    scr_a[:, :w],
                    sbuf[:, sp_off[j] : sp_off[j + 1]],
                    Act.Sigmoid,
                    bias=sig_bias[:, :],
                    scale=64.0,
                    accum_out=acc[:, nd + j : nd + j + 1],
                ).then_inc(act, 1)
            scalar.wait_ge(mm, 1)
            scalar.activation(red[0:1, :], ps.ap()[0:1, :], Act.Copy).then_inc(act, 1)
            scalar.wait_ge(act, ns + 1)
            scalar.dma_start(out=out_ext.ap()[:], in_=red[0:1, :]).then_inc(outd, 16)

        @block.tensor
        def _(tensor):
            tensor.wait_ge(dve, nd)
            tensor.wait_ge(act, ns)
            tensor.matmul(
                ps.ap()[0:1, :], ones_f, acc[:, :], start=True, stop=True
            ).then_inc(mm, 1)

    nc.finalize()
    return nc


def build_v5(shard=SHARD, x_cols=None, s_cols=None):
    """v4 + faster ramp and tail.

    - x DMAs on the sync HWDGE ring, s DMAs on the scalar HWDGE ring
      (two rings run concurrently; both streams ramp together)
    - tiny first chunks so DVE/ACT start ~3us earlier
    - no TensorE/PSUM final reduction: the idle sync engine ships the raw
      [128, nd+ns] f32 accumulator tile; host does the final 1.5 KB sum
    - only 3 engine programs (sync/vector/scalar) -> less preamble work
    """
    import concourse.bacc as bacc
    from concourse import mybir

    op = _get_sse_op()

    free = shard // P
    if x_cols is None:
        x_cols = [512, 1024, 2304, 2816, 2816, 2816, 2560, 1536]
    if s_cols is None:
        s_cols = [1024, 3072, 6144, 6144]
    if sum(x_cols) != free:
        x_cols = [free // 8] * 8
    if sum(s_cols) != free:
        s_cols = [free // 4] * 4
    assert sum(x_cols) == free and sum(s_cols) == free
    nd, ns = len(x_cols), len(s_cols)
    xp_off = [sum(x_cols[:i]) for i in range(nd + 1)]
    sp_off = [sum(s_cols[:i]) for i in range(ns + 1)]
    s_for_x = [next(j for j in range(ns) if sp_off[j + 1] >= xp_off[k + 1])
               for k in range(nd)]

    nc = bacc.Bacc("TRN2", target_bir_lowering=False)
    bf16 = mybir.dt.bfloat16
    f32 = mybir.dt.float32
    fp8 = mybir.dt.float8e4
    Act = mybir.ActivationFunctionType

    x_ext = nc.declare_dram_parameter("blast_scores", [shard], bf16, isOutput=False)
    s_ext = nc.declare_dram_parameter("stage_labels", [shard], fp8, isOutput=False)
    out_ext = nc.declare_dram_parameter("out", [P * (nd + ns)], f32, isOutput=True)

    x_v = x_ext.ap().rearrange("(p f) -> p f", p=P)
    s_v = s_ext.ap().rearrange("(p f) -> p f", p=P)
    out_v = out_ext.ap().rearrange("(p f) -> p f", p=P)

    xbuf = nc.alloc_sbuf_tensor("xbuf", [P, free], bf16).ap()
    sbuf = nc.alloc_sbuf_tensor("sbuf", [P, free], fp8).ap()
    scr_d = nc.alloc_sbuf_tensor("scr_d", [P, max(x_cols)], bf16).ap()
    scr_a = nc.alloc_sbuf_tensor("scr_a", [P, max(s_cols)], fp8).ap()
    acc = nc.alloc_sbuf_tensor("acc", [P, nd + ns], f32).ap()
    sig_bias = nc.alloc_sbuf_tensor("sig_bias", [P, 1], f32).ap()

    from contextlib import ExitStack

    with ExitStack() as ctx:
        dx = [ctx.enter_context(nc.semaphore(f"dx{i}")) for i in range(nd)]
        ds = [ctx.enter_context(nc.semaphore(f"ds{j}")) for j in range(ns)]
        dve = ctx.enter_context(nc.semaphore("dve"))
        act = ctx.enter_context(nc.semaphore("act"))
        outd = ctx.enter_context(nc.semaphore("outd"))
        bias_rdy = ctx.enter_context(nc.semaphore("bias_rdy"))
        block = ctx.enter_context(nc.Block())

        @block.sync
        def _(sync):
            for i in range(nd):
                sync.dma_start(
                    out=xbuf[:, xp_off[i] : xp_off[i + 1]],
                    in_=x_v[:, xp_off[i] : xp_off[i + 1]],
                ).then_inc(dx[i], 16)
            sync.wait_ge(dve, nd)
            sync.wait_ge(act, ns)
            sync.dma_start(out=out_v[:, :], in_=acc[:, :]).then_inc(outd, 16)

        @block.vector
        def _(vector):
            vector.memset(sig_bias[:, :], -480.0).then_inc(bias_rdy, 1)
            for k in range(nd):
                w = x_cols[k]
                vector.wait_ge(ds[s_for_x[k]], 16)
                vector.wait_ge(dx[k], 16)
                vector._custom_dve(
                    op,
                    out=scr_d[:, :w],
                    in0=xbuf[:, xp_off[k] : xp_off[k + 1]],
                    in1=sbuf[:, xp_off[k] : xp_off[k + 1]],
                    s0=1.75,
                    s1=8.0,
                    imm2=8.0,
                    accum_out=acc[:, k : k + 1],
                ).then_inc(dve, 1)

        @block.scalar
        def _(scalar):
            for j in range(ns):
                scalar.dma_start(
                    out=sbuf[:, sp_off[j] : sp_off[j + 1]],
                    in_=s_v[:, sp_off[j] : sp_off[j + 1]],
                ).then_inc(ds[j], 16)
            # warmup: pull the sigmoid table in while the first DMA lands
            scalar.activation(scr_a[:, 0:1], scr_a[:, 0:1], Act.Sigmoid)
            scalar.wait_ge(bias_rdy, 1)
            for j in range(ns):
                w = s_cols[j]
                scalar.wait_ge(ds[j], 16)
                scalar.activation(
                    scr_a[:, :w],
                    sbuf[:, sp_off[j] : sp_off[j + 1]],
                    Act.Sigmoid,
                    bias=sig_bias[:, :],
                    scale=64.0,
                    accum_out=acc[:, nd + j : nd + j + 1],
                ).then_inc(act, 1)

    nc.finalize()
    return nc


def build_v6(shard=SHARD, x_cols=None, s_cols=None):
    """v5 + both inputs staged fp8e3 (e3m4): 4 MB/core HBM traffic.

    The custom DVE op runs at 1x regardless of src dtype, so fp8 scores are
    free on the compute side (rel err ~2e-5 vs 7e-7 at bf16 -- gate is 2e-2).
    With DMA (~13 us) far under DVE (~18 us), a single sync-ring stream
    ordered by consumption keeps DVE gapless from the first tile.
    """
    import concourse.bacc as bacc
    from concourse import mybir

    op = _get_sse_op()

    free = shard // P
    if x_cols is None:
        x_cols = [512, 2048, 3072, 3584, 3584, 3584]
    if s_cols is None:
        s_cols = [2048, 4096, 5120, 5120]
    if sum(x_cols) != free:
        x_cols = [free // 8] * 8
    if sum(s_cols) != free:
        s_cols = [free // 4] * 4
    assert sum(x_cols) == free and sum(s_cols) == free
    nd, ns = len(x_cols), len(s_cols)
    xp_off = [sum(x_cols[:i]) for i in range(nd + 1)]
    sp_off = [sum(s_cols[:i]) for i in range(ns + 1)]
    s_for_x = [next(j for j in range(ns) if sp_off[j + 1] >= xp_off[k + 1])
               for k in range(nd)]

    nc = bacc.Bacc("TRN2", target_bir_lowering=False)
    f32 = mybir.dt.float32
    fp8 = mybir.dt.float8e3
    Act = mybir.ActivationFunctionType

    x_ext = nc.declare_dram_parameter("blast_scores", [shard], fp8, isOutput=False)
    s_ext = nc.declare_dram_parameter("stage_labels", [shard], fp8, isOutput=False)
    out_ext = nc.declare_dram_parameter("out", [P * (nd + ns)], f32, isOutput=True)

    x_v = x_ext.ap().rearrange("(p f) -> p f", p=P)
    s_v = s_ext.ap().rearrange("(p f) -> p f", p=P)
    out_v = out_ext.ap().rearrange("(p f) -> p f", p=P)

    xbuf = nc.alloc_sbuf_tensor("xbuf", [P, free], fp8).ap()
    sbuf = nc.alloc_sbuf_tensor("sbuf", [P, free], fp8).ap()
    scr_d = nc.alloc_sbuf_tensor("scr_d", [P, max(x_cols)], mybir.dt.bfloat16).ap()
    scr_a = nc.alloc_sbuf_tensor("scr_a", [P, max(s_cols)], fp8).ap()
    acc = nc.alloc_sbuf_tensor("acc", [P, nd + ns], f32).ap()
    sig_bias = nc.alloc_sbuf_tensor("sig_bias", [P, 1], f32).ap()

    from contextlib import ExitStack

    with ExitStack() as ctx:
        dx = [ctx.enter_context(nc.semaphore(f"dx{i}")) for i in range(nd)]
        ds = [ctx.enter_context(nc.semaphore(f"ds{j}")) for j in range(ns)]
        dve = ctx.enter_context(nc.semaphore("dve"))
        act = ctx.enter_context(nc.semaphore("act"))
        outd = ctx.enter_context(nc.semaphore("outd"))
        bias_rdy = ctx.enter_context(nc.semaphore("bias_rdy"))
        block = ctx.enter_context(nc.Block())

        # single-ring issue order: each s chunk right before the first x
        # chunk that needs it; s0 first so ACT can start asap
        issue = []
        xi = 0
        for j in range(ns):
            issue.append(("s", j))
            while xi < nd and s_for_x[xi] <= j:
                issue.append(("x", xi)); xi += 1

        @block.sync
        def _(sync):
            for kind, i in issue:
                if kind == "s":
                    sync.dma_start(
                        out=sbuf[:, sp_off[i] : sp_off[i + 1]],
                        in_=s_v[:, sp_off[i] : sp_off[i + 1]],
                    ).then_inc(ds[i], 16)
                else:
                    sync.dma_start(
                        out=xbuf[:, xp_off[i] : xp_off[i + 1]],
                        in_=x_v[:, xp_off[i] : xp_off[i + 1]],
                    ).then_inc(dx[i], 16)
            sync.wait_ge(dve, nd)
            sync.wait_ge(act, ns)
            sync.dma_start(out=out_v[:, :], in_=acc[:, :]).then_inc(outd, 16)

        @block.vector
        def _(vector):
            vector.memset(sig_bias[:, :], -480.0).then_inc(bias_rdy, 1)
            for k in range(nd):
                w = x_cols[k]
                vector.wait_ge(ds[s_for_x[k]], 16)
                vector.wait_ge(dx[k], 16)
                vector._custom_dve(
                    op,
                    out=scr_d[:, :w],
                    in0=xbuf[:, xp_off[k] : xp_off[k + 1]],
                    in1=sbuf[:, xp_off[k] : xp_off[k + 1]],
                    s0=1.75,
                    s1=8.0,
                    imm2=8.0,
                    accum_out=acc[:, k : k + 1],
                ).then_inc(dve, 1)

        @block.scalar
        def _(scalar):
            # warmup first: sigmoid tables load while the first DMAs land
            scalar.activation(scr_a[:, 0:1], scr_a[:, 0:1], Act.Sigmoid)
            scalar.wait_ge(bias_rdy, 1)
            for j in range(ns):
                w = s_cols[j]
                scalar.wait_ge(ds[j], 16)
                scalar.activation(
                    scr_a[:, :w],
                    sbuf[:, sp_off[j] : sp_off[j + 1]],
                    Act.Sigmoid,
                    bias=sig_bias[:, :],
                    scale=64.0,
                    accum_out=acc[:, nd + j : nd + j + 1],
                ).then_inc(act, 1)

    nc.finalize()
    return nc


def build_v7(shard=SHARD, x_cols=None, s_cols=None):
    """v6 + three parallel DMA paths so DVE is never starved.

    sync HWDGE ring: x chunks only; gpsimd SWDGE ring: s chunks; scalar:
    pure ACT (tables load at t0).  Both inputs fp8e3 (4 MB/core).
    """
    import concourse.bacc as bacc
    from concourse import mybir

    op = _get_sse_op()

    free = shard // P
    if x_cols is None:
        x_cols = [512, 1536, 2048, 2048, 2560, 2560, 2560, 2560]
    if s_cols is None:
        s_cols = [4096, 6144, 6144]
    if sum(x_cols) != free:
        x_cols = [free // 8] * 8
    if sum(s_cols) != free:
        s_cols = [free // 4] * 4
    assert sum(x_cols) == free and sum(s_cols) == free
    nd, ns = len(x_cols), len(s_cols)
    xp_off = [sum(x_cols[:i]) for i in range(nd + 1)]
    sp_off = [sum(s_cols[:i]) for i in range(ns + 1)]
    s_for_x = [next(j for j in range(ns) if sp_off[j + 1] >= xp_off[k + 1])
               for k in range(nd)]

    nc = bacc.Bacc("TRN2", target_bir_lowering=False)
    f32 = mybir.dt.float32
    fp8 = mybir.dt.float8e3
    Act = mybir.ActivationFunctionType

    x_ext = nc.declare_dram_parameter("blast_scores", [shard], fp8, isOutput=False)
    s_ext = nc.declare_dram_parameter("stage_labels", [shard], fp8, isOutput=False)
    out_ext = nc.declare_dram_parameter("out", [P * (nd + ns)], f32, isOutput=True)

    x_v = x_ext.ap().rearrange("(p f) -> p f", p=P)
    s_v = s_ext.ap().rearrange("(p f) -> p f", p=P)
    out_v = out_ext.ap().rearrange("(p f) -> p f", p=P)

    xbuf = nc.alloc_sbuf_tensor("xbuf", [P, free], fp8).ap()
    sbuf = nc.alloc_sbuf_tensor("sbuf", [P, free], fp8).ap()
    scr_d = nc.alloc_sbuf_tensor("scr_d", [P, max(x_cols)], mybir.dt.bfloat16).ap()
    scr_a = nc.alloc_sbuf_tensor("scr_a", [P, max(s_cols)], fp8).ap()
    acc = nc.alloc_sbuf_tensor("acc", [P, nd + ns], f32).ap()
    sig_bias = nc.alloc_sbuf_tensor("sig_bias", [P, 1], f32).ap()

    from contextlib import ExitStack

    with ExitStack() as ctx:
        dx = [ctx.enter_context(nc.semaphore(f"dx{i}")) for i in range(nd)]
        ds = [ctx.enter_context(nc.semaphore(f"ds{j}")) for j in range(ns)]
        dve = ctx.enter_context(nc.semaphore("dve"))
        act = ctx.enter_context(nc.semaphore("act"))
        outd = ctx.enter_context(nc.semaphore("outd"))
        bias_rdy = ctx.enter_context(nc.semaphore("bias_rdy"))
        block = ctx.enter_context(nc.Block())

        @block.sync
        def _(sync):
            for i in range(nd):
                sync.dma_start(
                    out=xbuf[:, xp_off[i] : xp_off[i + 1]],
                    in_=x_v[:, xp_off[i] : xp_off[i + 1]],
                ).then_inc(dx[i], 16)
            sync.wait_ge(dve, nd)
            sync.wait_ge(act, ns)
            sync.dma_start(out=out_v[:, :], in_=acc[:, :]).then_inc(outd, 16)

        @block.gpsimd
        def _(gpsimd):
            gpsimd.memset(sig_bias[:, :], -480.0).then_inc(bias_rdy, 1)
            for j in range(ns):
                gpsimd.dma_start(
                    out=sbuf[:, sp_off[j] : sp_off[j + 1]],
                    in_=s_v[:, sp_off[j] : sp_off[j + 1]],
                ).then_inc(ds[j], 16)

        @block.vector
        def _(vector):
            for k in range(nd):
                w = x_cols[k]
                vector.wait_ge(ds[s_for_x[k]], 16)
                vector.wait_ge(dx[k], 16)
                vector._custom_dve(
                    op,
                    out=scr_d[:, :w],
                    in0=xbuf[:, xp_off[k] : xp_off[k + 1]],
                    in1=sbuf[:, xp_off[k] : xp_off[k + 1]],
                    s0=1.75,
                    s1=8.0,
                    imm2=8.0,
                    accum_out=acc[:, k : k + 1],
                ).then_inc(dve, 1)

        @block.scalar
        def _(scalar):
            # warmup first: sigmoid tables load while the first DMAs land
            scalar.activation(scr_a[:, 0:1], scr_a[:, 0:1], Act.Sigmoid)
            scalar.wait_ge(bias_rdy, 1)
            for j in range(ns):
                w = s_cols[j]
                scalar.wait_ge(ds[j], 16)
                scalar.activation(
                    scr_a[:, :w],
                    sbuf[:, sp_off[j] : sp_off[j + 1]],
                    Act.Sigmoid,
                    bias=sig_bias[:, :],
                    scale=64.0,
                    accum_out=acc[:, nd + j : nd + j + 1],
                ).then_inc(act, 1)

    nc.finalize()
    return nc


V8_COLS = [256, 512, 768, 1024, 1536, 2048, 2304, 2560, 2688, 2688]
V8_ACT_GROUPS = 2


def build_v8(shard=SHARD, cols=None, act_groups=None):
    """Interleaved single-stream design.

    Host interleaves x and s into one fp8e3 array [x0,s0,x1,s1,...]; each
    DMA chunk carries both tensors for its span, so one semaphore gates
    both consumers and the x/s delivery ratio is always right.  Chunks
    alternate between the sync and scalar HWDGE rings (2 in flight).
    DVE reads stride-2 views (1x mode is stride-agnostic); ACT sigmoid
    reads the stride-2 s view.
    """
    import concourse.bacc as bacc
    from concourse import mybir

    op = _get_sse_op()

    free = shard // P  # pairs per partition
    if cols is None:
        cols = list(V8_COLS)
    if act_groups is None:
        act_groups = V8_ACT_GROUPS
    if sum(cols) != free:
        cols = [free // 8] * 8
    assert sum(cols) == free
    nd = len(cols)
    off = [sum(cols[:i]) for i in range(nd + 1)]
    # ACT op j covers chunks [j*act_groups, (j+1)*act_groups)
    assert nd % act_groups == 0
    ns = nd // act_groups

    nc = bacc.Bacc("TRN2", target_bir_lowering=False)
    f32 = mybir.dt.float32
    fp8 = mybir.dt.float8e3
    Act = mybir.ActivationFunctionType

    xs_ext = nc.declare_dram_parameter("xs", [2 * shard], fp8, isOutput=False)
    out_ext = nc.declare_dram_parameter("out", [P * (nd + ns)], f32, isOutput=True)

    xs_v = xs_ext.ap().rearrange("(p f two) -> p f two", p=P, two=2)
    out_v = out_ext.ap().rearrange("(p f) -> p f", p=P)

    ibuf = nc.alloc_sbuf_tensor("ibuf", [P, free, 2], fp8).ap()
    x_view = ibuf[:, :, 0]
    s_view = ibuf[:, :, 1]
    scr_d = nc.alloc_sbuf_tensor("scr_d", [P, max(cols)], mybir.dt.bfloat16).ap()
    scr_a = nc.alloc_sbuf_tensor("scr_a", [P, act_groups * max(cols)], fp8).ap()
    acc = nc.alloc_sbuf_tensor("acc", [P, nd + ns], f32).ap()
    sig_bias = nc.alloc_sbuf_tensor("sig_bias", [P, 1], f32).ap()

    from contextlib import ExitStack

    with ExitStack() as ctx:
        dc = [ctx.enter_context(nc.semaphore(f"dc{i}")) for i in range(nd)]
        dve = ctx.enter_context(nc.semaphore("dve"))
        act = ctx.enter_context(nc.semaphore("act"))
        outd = ctx.enter_context(nc.semaphore("outd"))
        bias_rdy = ctx.enter_context(nc.semaphore("bias_rdy"))
        block = ctx.enter_context(nc.Block())

        @block.sync
        def _(sync):
            for i in range(0, nd, 2):
                sync.dma_start(
                    out=ibuf[:, off[i] : off[i + 1], :],
                    in_=xs_v[:, off[i] : off[i + 1], :],
                ).then_inc(dc[i], 16)
            sync.wait_ge(dve, nd)
            sync.wait_ge(act, ns)
            sync.dma_start(out=out_v[:, :], in_=acc[:, :]).then_inc(outd, 16)

        @block.vector
        def _(vector):
            vector.memset(sig_bias[:, :], -480.0).then_inc(bias_rdy, 1)
            for k in range(nd):
                w = cols[k]
                vector.wait_ge(dc[k], 16)
                vector._custom_dve(
                    op,
                    out=scr_d[:, :w],
                    in0=x_view[:, off[k] : off[k + 1]],
                    in1=s_view[:, off[k] : off[k + 1]],
                    s0=1.75,
                    s1=8.0,
                    imm2=8.0,
                    accum_out=acc[:, k : k + 1],
                ).then_inc(dve, 1)

        @block.scalar
        def _(scalar):
            for i in range(1, nd, 2):
                scalar.dma_start(
                    out=ibuf[:, off[i] : off[i + 1], :],
                    in_=xs_v[:, off[i] : off[i + 1], :],
                ).then_inc(dc[i], 16)
            # warmup: sigmoid tables load while the first chunks land
            scalar.activation(scr_a[:, 0:1], scr_a[:, 0:1], Act.Sigmoid)
            scalar.wait_ge(bias_rdy, 1)
            for j in range(ns):
                lo, hi = off[j * act_groups], off[(j + 1) * act_groups]
                for g in range(j * act_groups, (j + 1) * act_groups):
                    scalar.wait_ge(dc[g], 16)
                scalar.activation(
                    scr_a[:, : hi - lo],
                    s_view[:, lo:hi],
                    Act.Sigmoid,
                    bias=sig_bias[:, :],
                    scale=64.0,
                    accum_out=acc[:, nd + j : nd + j + 1],
                ).then_inc(act, 1)

    nc.finalize()
    return nc


def _to_bf16(a):
    import ml_dtypes

    return np.ascontiguousarray(a.astype(ml_dtypes.bfloat16))


def _to_fp8(a):
    import ml_dtypes

    return np.ascontiguousarray(a.astype(np.float32).astype(ml_dtypes.float8_e4m3fn))


def _to_fp8e3(a):
    import ml_dtypes

    return np.ascontiguousarray(a.astype(np.float32).astype(ml_dtypes.float8_e3m4))


def run(x, s, variant="v3", **spmd_kwargs):
    """Shard, run on 8 cores, host-reduce. Returns (loss, BassKernelResults)."""
    from concourse.bass_utils import run_bass_kernel_spmd

    if variant not in _NC_CACHE:
        if variant == "raw":
            _NC_CACHE[variant] = build_raw()
        elif variant == "v3":
            _NC_CACHE[variant] = build_v3()
        elif variant == "v4":
            _NC_CACHE[variant] = build_v4()
        elif variant == "v5":
            _NC_CACHE[variant] = build_v5()
        elif variant == "v6":
            _NC_CACHE[variant] = build_v6()
        elif variant == "v7":
            _NC_CACHE[variant] = build_v7()
        elif variant == "v8":
            _NC_CACHE[variant] = build_v8()
        else:
            raise ValueError(variant)
    nc = _NC_CACHE[variant]

    if variant == "v8":
        import ml_dtypes

        inter = np.empty(2 * B, dtype=ml_dtypes.float8_e3m4)
        inter[0::2] = x.astype(ml_dtypes.float8_e3m4)
        inter[1::2] = s.astype(np.float32).astype(ml_dtypes.float8_e3m4)
        in_maps = [
            {"xs": inter[i * 2 * SHARD : (i + 1) * 2 * SHARD]}
            for i in range(N_CORES)
        ]
        res = run_bass_kernel_spmd(
            nc, in_maps, core_ids=list(range(N_CORES)), **spmd_kwargs
        )
        nd = len(V8_COLS)
        ns = nd // V8_ACT_GROUPS
        cnt = 0.0
        sse = 0.0
        for r in res.results:
            o = r["out"].astype(np.float64).reshape(P, nd + ns)
            sse += o[:, :nd].sum() / 3.0625
            cnt += o[:, nd:].sum()
        val = sse / max(cnt, 1.0) if cnt > 0 else 0.0
        return np.asarray(val, dtype=np.float32), res

    if variant == "raw":
        xs, ss = x, s
    elif variant in ("v6", "v7"):
        xs, ss = _to_fp8e3(x), _to_fp8e3(s)
    elif variant in ("v4", "v5"):
        xs, ss = _to_bf16(x), _to_fp8(s)
    else:
        xs, ss = _to_bf16(x), _to_bf16(s)

    in_maps = [
        {
            "blast_scores": xs[i * SHARD : (i + 1) * SHARD],
            "stage_labels": ss[i * SHARD : (i + 1) * SHARD],
        }
        for i in range(N_CORES)
    ]
    res = run_bass_kernel_spmd(nc, in_maps, core_ids=list(range(N_CORES)), **spmd_kwargs)

    cnt = 0.0
    sse = 0.0
    for r in res.results:
        o = r["out"].astype(np.float64)
        if variant == "raw":
            o = o.reshape(2, -1)
            cnt += o[0].sum()
            sse += o[1].sum()
        elif variant == "v4":
            sse += o[:8].sum() / 3.0625  # undo the 1.75^2 prescale
            cnt += o[8:].sum()
        elif variant == "v5":
            o = o.reshape(P, 12)
            sse += o[:, :8].sum() / 3.0625
            cnt += o[:, 8:].sum()
        elif variant == "v6":
            o = o.reshape(P, 10)
            sse += o[:, :6].sum() / 3.0625
            cnt += o[:, 6:].sum()
        elif variant == "v7":
            o = o.reshape(P, 11)
            sse += o[:, :8].sum() / 3.0625
            cnt += o[:, 8:].sum()
        else:
            cnt += o[:512].sum()
            sse += o[512:].sum()
    val = sse / max(cnt, 1.0) if cnt > 0 else 0.0
    return np.asarray(val, dtype=np.float32), res


def kernel(**inputs):
    x = np.ascontiguousarray(np.asarray(inputs["blast_scores"], dtype=np.float32))
    s = np.ascontiguousarray(np.asarray(inputs["stage_labels"], dtype=np.int32))
    assert x.shape == (B,) and s.shape == (B,)
    return run(x, s)[0]

